# revision 36
# baseline (speedup 1.0000x reference)
"""Trainium2 Bass kernel for a dense pre-LN transformer block.

Shapes (hardcoded): B=2, T=2048, C=768, H=12, D=64, hidden=3072, fp32 I/O.

Strategy (8 NeuronCores, two SPMD launches, host glue between them):
  Launch 1 (attention): core = (batch b in {0,1}) x (head-group of 3 heads).
    Host precomputes LN1(x) (gain/bias applied), transposes it to
    feature-major and quantizes to fp8-e4m3.  Each core: Q/K/V projections
    for its 3 heads as fp8 DoubleRow matmuls (256-row contraction per
    instruction), causal attention in S^T = K @ Q^T layout (keys on
    partitions, so the softmax matrix feeds the A@V matmul as the
    stationary operand).  exp() runs on ScalarE over [128, 3, w] groups
    (all 3 heads of a key-block row in one instruction).  Softmax uses no
    max-subtraction (scores ~ N(0, 0.3)); the denominator comes free from
    a ones-column appended to V.  Output: per-head UNNORMALIZED numerator
    + denominator, bf16; the host divides, assembles heads, adds the
    residual (x_mid = x + attn).
  Launch 2 (MLP): core = 512 contiguous tokens of the flattened [4096, C].
    Host precomputes LN2(x_mid), transposed, bf16.  Device: MLP1 (bf16)
    -> relu+bias on ScalarE -> MLP2 (bf16, token-major output) -> bf16
    out.  Host adds x_mid + b_proj.

All heavy math (all matmuls, exp/softmax, relu) runs on device; the host
does input preprocessing (layernorms over inputs / the inter-launch
residual state), sharding, and output assembly.
"""

import os
import sys
import math

for _p in ("/opt/trn_rl_repo", "/root/.axon_site/_ro/trn_rl_repo"):
    if _p not in sys.path and os.path.isdir(_p):
        sys.path.insert(0, _p)

import numpy as np
import ml_dtypes

import concourse.bass as bass
import concourse.mybir as mybir
import concourse.tile as tile
from concourse import bacc
from concourse import bass_utils

BF16 = mybir.dt.bfloat16
F32 = mybir.dt.float32
FP8 = mybir.dt.float8e4
AF = mybir.ActivationFunctionType
DR = mybir.MatmulPerfMode.DoubleRow

B, T, C, H, D = 2, 2048, 768, 12, 64
HID = 4 * C                     # 3072
EPS = 1e-5
SCALE = 1.0 / math.sqrt(C)      # reference scales scores by 1/sqrt(C)
NC_PER_B = 4                    # cores per batch in launch 1
HG = H // NC_PER_B              # heads per core (3)
P = 128
CCH = C // P                    # 6 feature chunks
TBLK = T // P                   # 16 token blocks of 128
ROWS2 = (B * T) // 8            # 512 tokens per core in launch 2
HCH = HID // P                  # 24 hidden chunks
OW = HG * 65                    # 195: per-token attn payload (num|den x 3)
OWP = 256                       # padded to 512B rows for clean DMA
USE_DR_AV = bool(int(os.environ.get("USE_DR_AV", "1")))
WARM1 = 20                      # PE p-state warmup matmuls (launch 1)
WARM2 = 16                      # PE p-state warmup matmuls (launch 2)

_cache = {}


def build_attn():
    """LN'd input (host) -> QKV proj (fp8 DR) -> causal attention."""
    nc = bacc.Bacc("TRN2", target_bir_lowering=False, debug=False,
                   num_devices=8)
    xhT = nc.dram_tensor("xhT", [C, T], FP8, kind="ExternalInput")
    # wall cols: K01 0:128 | Kh2 128:192 | Q01 192:320 | Qh2 320:384 |
    #            I128 384:512 (fp8 identity, rows 0:128) | V 512:704
    wall = nc.dram_tensor("wall", [C, 704], FP8, kind="ExternalInput")
    oO = nc.dram_tensor("oO", [T, OWP], BF16, kind="ExternalOutput")
    dbg = os.environ.get("DEBUG_PT")
    if dbg:
        dpt = nc.dram_tensor("dpt", [P, 4 * HG * 2 * 1024], FP8,
                             kind="ExternalOutput")
        dva = nc.dram_tensor("dva", [P, 8 * HG * 2 * 65], FP8,
                             kind="ExternalOutput")

    with tile.TileContext(nc) as tc:
        with (
            tc.tile_pool(name="pers", bufs=1) as pers,
            tc.tile_pool(name="aux", bufs=2, space="PSUM") as aux,
        ):
            # --- PE warmup: absorb the p-state ramp during the DMA wait ---
            wa = pers.tile([P, 512], BF16)
            nc.vector.memset(wa, 0.0)
            for i in range(WARM1):
                wacc = aux.tile([P, 512], F32, tag="aux")
                w = 512 if i < 8 else 64
                nc.tensor.matmul(wacc[:, 0:w], wa[:, 0:P], wa[:, 0:w],
                                 start=True, stop=True)

            # --- persistent SBUF (QK weights first, then first token
            # quarter, so the first score group starts early) ---
            wall_t = pers.tile([P, CCH, 704], FP8)
            wall_r = wall.rearrange("(c p) f -> p c f", p=P)
            xh_t = pers.tile([P, CCH, T], FP8)
            xh_r = xhT.rearrange("(c p) t -> p c t", p=P)
            nc.sync.dma_start(wall_t[:, :, 0:512], wall_r[:, :, 0:512])
            nc.sync.dma_start(xh_t[:, :, 0:512], xh_r[:, :, 0:512])
            nc.sync.dma_start(wall_t[:, :, 512:704], wall_r[:, :, 512:704])
            for qq in range(1, 4):
                nc.sync.dma_start(xh_t[:, :, qq * 512:(qq + 1) * 512],
                                  xh_r[:, :, qq * 512:(qq + 1) * 512])

            # -240 strict-upper-triangle const: the causal mask is a
            # single psum-accumulated ident^T @ negU, emitted BEFORE the
            # diagonal score matmul so the PE can run it early (it has no
            # data deps) and ScalarE never waits on it.
            # exp((s - 240) * SCALE) < 3e-4 relative leak, only visible in
            # the first few tokens' denominators; -240 is fp8e4-exact.
            ident = wall_t[:, 0, 384:512]
            negU = pers.tile([P, P], FP8)
            nc.gpsimd.memset(negU, 0.0)
            nc.gpsimd.affine_select(
                out=negU, in_=negU, compare_op=mybir.AluOpType.is_ge,
                fill=-240.0, base=0, pattern=[[1, P]], channel_multiplier=-1)

            # V in fp8 with a ones column (denominator), paired key blocks
            # so AV runs as DoubleRow over 256-key contractions
            vaug = pers.tile([P, TBLK // 2, HG, 2, 65], FP8)
            nc.vector.memset(vaug[:, :, :, :, 64:65], 1.0)

            # QKT[p, s, 0, t] = Q features, QKT[p, s, 1, t] = K features;
            # head h lives at partitions 64*(h%2).. with slot s = h//2, so
            # each head's Q and K share a physical partition range (the
            # scores matmul requires equal base partitions).
            QKT = pers.tile([P, 2, 2, T], BF16)
            # probs (exp output) in fp8, paired-key-block layout; separate
            # arrays per query half so tt1 groups never overwrite blocks
            # the tt0 AVs still read
            pt0 = pers.tile([P, 4, HG, 2, 1024], FP8)
            pt1 = pers.tile([P, 8, HG, 2, 1024], FP8)
            o_store = pers.tile([P, TBLK, OWP], BF16)

            # wall col groups: K01@0:128, Kh2@128:192, Q01@192:320,
            # Qh2@320:384, V@384:576
            QK_GROUPS = [  # (col0, width, slot, qk)
                (192, P, 0, 0), (0, P, 0, 1),
                (320, 64, 1, 0), (128, 64, 1, 1),
            ]

            def qk_proj(tch, order=(0, 1, 2, 3)):
                for gi in order:
                    col0, gw, sl, qk = QK_GROUPS[gi]
                    acc = aux.tile([P, 512], F32, tag="aux")
                    for k in range(3):
                        nc.tensor.matmul(
                            acc[0:gw],
                            wall_t[:, 2 * k:2 * k + 2, col0:col0 + gw],
                            xh_t[:, 2 * k:2 * k + 2,
                                 tch * 512:(tch + 1) * 512],
                            start=(k == 0), stop=(k == 2), perf_mode=DR)
                    nc.vector.tensor_copy(
                        QKT[0:gw, sl, qk, tch * 512:(tch + 1) * 512],
                        acc[0:gw])

            def v_proj(ob):
                acc = aux.tile([P, 512], F32, tag="aux")
                for k in range(3):
                    nc.tensor.matmul(
                        acc[:, 0:192],
                        xh_t[:, 2 * k:2 * k + 2, ob * P:(ob + 1) * P],
                        wall_t[:, 2 * k:2 * k + 2, 512:704],
                        start=(k == 0), stop=(k == 2), perf_mode=DR)
                nc.vector.tensor_copy(
                    vaug[:, ob // 2, :, ob % 2, 0:64],
                    acc[:, 0:192].rearrange("p (h d) -> p h d", h=HG))

            qk_proj(0, order=(3, 2, 1, 0))
            qk_proj(1, order=(0, 1, 2, 3))

            # deferred work to interleave into the score loops (PE has
            # slack while ScalarE exp is the bottleneck); kept small per
            # item so a pop never delays the next score matmuls by much
            deferred = [
                lambda: qk_proj(2, order=(1, 0)),
                lambda: qk_proj(2, order=(3, 2)),
                lambda: qk_proj(3, order=(1, 0)),
                lambda: qk_proj(3, order=(3, 2)),
            ] + [lambda ob=ob: v_proj(ob) for ob in range(8, 16)]

            # Two independent single-buffered score pools (heads 0-1 / head
            # 2) so PE fills one while ScalarE exps the other.
            with (
                tc.tile_pool(name="scA", bufs=1, space="PSUM") as scpA,
                tc.tile_pool(name="scB", bufs=1, space="PSUM") as scpB,
            ):
                o_r = oO.rearrange("(o p) f -> p o f", p=P)

                def scores(sc, hs, tt, kb, off, w, diag):
                    for i, h in enumerate(hs):
                        sl, hsel = divmod(h, 2)
                        pb = 64 * hsel
                        s = 0
                        if diag:
                            # diagonal 128 columns: their own psum group;
                            # masks first (dep-free, hoistable), scores close
                            mdst = sc[:, i, off:off + P] \
                                if len(hs) > 1 else sc[:, off:off + P]
                            q0 = tt * 1024 + off
                            nc.tensor.matmul(
                                mdst, ident, negU, start=True,
                                stop=False, skip_group_check=True)
                            nc.tensor.matmul(
                                mdst,
                                QKT[pb:pb + 64, sl, 1, kb * P:(kb + 1) * P],
                                QKT[pb:pb + 64, sl, 0, q0:q0 + P],
                                start=False, stop=True,
                                skip_group_check=True)
                            s = P
                        while s < w:
                            ww = min(512, w - s)
                            q0 = tt * 1024 + off + s
                            dst = sc[:, i, off + s:off + s + ww] \
                                if len(hs) > 1 else sc[:, off + s:off + s + ww]
                            nc.tensor.matmul(
                                dst,
                                QKT[pb:pb + 64, sl, 1, kb * P:(kb + 1) * P],
                                QKT[pb:pb + 64, sl, 0, q0:q0 + ww],
                                start=True, stop=True)
                            s += ww

                def av_store(gq, oacc):
                    nc.vector.tensor_copy(
                        o_store[:, gq, 0:OW], oacc[:, 0:OW])
                    if gq == 14:
                        nc.sync.dma_start(o_r[:, 12:15, :],
                                          o_store[:, 12:15, :])
                    elif gq == 15:
                        nc.sync.dma_start(o_r[:, 15:16, :],
                                          o_store[:, 15:16, :])
                    elif gq % 4 == 3:
                        nc.sync.dma_start(
                            o_r[:, gq - 3:gq + 1, :],
                            o_store[:, gq - 3:gq + 1, :])

                def av_mms(gq, oacc, k2s, last):
                    # paired key blocks run as fp8 DoubleRow (256-key
                    # contraction per matmul); stragglers as single fp8.
                    gl = gq % 8
                    pta = pt0 if gq < 8 else pt1
                    csl = slice(gl * P, (gl + 1) * P)
                    k2s = list(k2s)
                    items, i = [], 0
                    while i < len(k2s):
                        k2 = k2s[i]
                        if USE_DR_AV and k2 % 2 == 0 and i + 1 < len(k2s) \
                                and k2s[i + 1] == k2 + 1:
                            items.append((True, k2 // 2)); i += 2
                        else:
                            items.append((False, k2)); i += 1
                    first_grp = k2s[0] == 0
                    for j, (pair, idx) in enumerate(items):
                        for h in range(HG):
                            st = first_grp and j == 0 and h == 0
                            sp = last and j == len(items) - 1 and h == HG - 1
                            if pair:
                                nc.tensor.matmul(
                                    oacc[:, h * 65:(h + 1) * 65],
                                    pta[:, idx, h, :, csl],
                                    vaug[:, idx, h, :, :],
                                    start=st, stop=sp, perf_mode=DR,
                                    skip_group_check=True)
                            else:
                                nc.tensor.matmul(
                                    oacc[:, h * 65:(h + 1) * 65],
                                    pta[:, idx // 2, h, idx % 2, csl],
                                    vaug[:, idx // 2, h, idx % 2, :],
                                    start=st, stop=sp,
                                    skip_group_check=True)

                def group(tt, kb, fill=0, split=False, b_first=True):
                    off = max(0, P * kb - 1024 * tt)
                    diag = P * kb >= 1024 * tt
                    # (off, width) segments; splitting the first groups at
                    # q=512 lets the exp stream start as soon as the first
                    # xh DMA quarter lands (segment b's data arrives while
                    # ScalarE works on segment a)
                    segs = [(off, 512 - off), (512, 512)] if split \
                        else [(off, 1024 - off)]
                    pta = pt0 if tt == 0 else pt1
                    dstA = pta[:, kb // 2, 0:2, kb % 2, :]
                    dstB = pta[:, kb // 2, 2, kb % 2, :]
                    scA = scpA.tile([P, 2, 1024], F32, tag="scA")
                    scB = scpB.tile([P, 1024], F32, tag="scB")
                    def emit_a(so, sw, si):
                        scores(scA, (0, 1), tt, kb, so, sw, diag and si == 0)
                        nc.scalar.activation(
                            dstA[:, :, so:so + sw], scA[:, :, so:so + sw],
                            AF.Exp, scale=SCALE)

                    def emit_b(so, sw, si):
                        scores(scB, (2,), tt, kb, so, sw, diag and si == 0)
                        nc.scalar.activation(
                            dstB[:, so:so + sw], scB[:, so:so + sw],
                            AF.Exp, scale=SCALE)

                    for si, (so, sw) in enumerate(segs):
                        if b_first:
                            emit_b(so, sw, si)
                            emit_a(so, sw, si)
                        else:
                            emit_a(so, sw, si)
                            emit_b(so, sw, si)
                    # PE filler (runs while ScalarE exps this group); emitted
                    # after the score matmuls so it can't delay them
                    for _ in range(fill):
                        if deferred:
                            deferred.pop(0)()

                def av_full(gq):
                    oacc = aux.tile([P, 512], F32, tag="aux")
                    av_mms(gq, oacc, range(gq + 1), True)
                    av_store(gq, oacc)

                # tt0: ascending kb; AV(gq) emitted one iteration late so
                # it runs inside ScalarE's exp window of the next group.
                # No deferred pops in the first 3 groups (that work needs
                # DMA quarters 3-4 and would stall the PE FIFO).
                for kb in range(8):
                    group(0, kb, fill=0 if kb < 3 else 1, split=(kb < 2),
                          b_first=True)
                    # early V projections land here, behind the first score
                    # matmuls instead of ahead of them in the PE FIFO
                    if 1 <= kb <= 4:
                        v_proj(2 * (kb - 1))
                        v_proj(2 * (kb - 1) + 1)
                    if kb >= 1:
                        av_full(kb - 1)
                # phase boundary: tt1's first group writes pt1, so it can
                # be emitted before the last tt0 AV still reading pt0
                group(1, 0, fill=1)
                av_full(7)

                # tt1: ascending kb, software-pipelined AV: part1 (all key
                # blocks except the diagonal) is emitted right after this
                # group's score matmuls and runs inside ScalarE's exp
                # window; the 3-matmul diagonal part2 + copy-out are emitted
                # after the NEXT group's scores so they never delay them.
                part_acc = {}
                for kb in range(1, 16):
                    group(1, kb, fill=1 if kb < 8 else 0)
                    if kb >= 9:
                        pa = part_acc.pop(kb - 1)
                        av_mms(kb - 1, pa, [kb - 1], True)
                        av_store(kb - 1, pa)
                    if kb >= 8:
                        oacc = aux.tile([P, 512], F32, tag="aux")
                        part_acc[kb] = oacc
                        av_mms(kb, oacc, range(kb), False)
                pa = part_acc.pop(15)
                av_mms(15, pa, [15], True)
                av_store(15, pa)
                if dbg:
                    nc.sync.dma_start(
                        dpt[:, :], pt0.rearrange("p a h j c -> p (a h j c)"))
                    nc.sync.dma_start(
                        dva[:, :], vaug.rearrange("p a h j c -> p (a h j c)"))
    nc.compile()
    return nc


def build_mlp():
    """Host-LN'd x_mid -> MLP1 -> relu -> MLP2 for 512 tokens per core.

    fp8 DoubleRow with power-of-2-scaled error-correction matmuls:
      MLP1 psum (x8 precision, 8x scaled) =
          x8 @ W8 + (x8/32) @ dW8 + dx8 @ W8
      where W8 = f8(8 w), dW8 = f8(256 (w - W8/8)), x8 = f8(h2),
      dx8 = f8(h2 - x8).  relu applies scale 1/8.  hid emitted twice by
      ScalarE: hid8 = f8(relu) and hid8c = f8(relu/8) (corr operand).
      MLP2 psum = hid8 @ Wp8 + hid8c @ dWp8  (8x scaled; host multiplies
      the output by 1/8, exact).
    """
    nc = bacc.Bacc("TRN2", target_bir_lowering=False, debug=False,
                   num_devices=8)
    xl8 = nc.dram_tensor("xl8", [C, ROWS2], FP8, kind="ExternalInput")
    xl8c = nc.dram_tensor("xl8c", [C, ROWS2], FP8, kind="ExternalInput")
    xl8d = nc.dram_tensor("xl8d", [C, ROWS2], FP8, kind="ExternalInput")
    wh8 = nc.dram_tensor("wh8", [C, HID], FP8, kind="ExternalInput")
    dwh8 = nc.dram_tensor("dwh8", [C, HID], FP8, kind="ExternalInput")
    wp8 = nc.dram_tensor("wp8", [HID, C], FP8, kind="ExternalInput")
    dwp8 = nc.dram_tensor("dwp8", [HID, C], FP8, kind="ExternalInput")
    bh = nc.dram_tensor("bh", [P, 2, HCH], F32, kind="ExternalInput")
    oq = nc.dram_tensor("oq", [ROWS2, C], BF16, kind="ExternalOutput")

    NO = ROWS2 // P  # 4 token sub-blocks
    with tile.TileContext(nc) as tc:
        with (
            tc.tile_pool(name="pers", bufs=1) as pers,
            tc.tile_pool(name="psA", bufs=2, space="PSUM") as psA,
            tc.tile_pool(name="psB", bufs=1, space="PSUM") as psB,
        ):
            # PE warmup during the initial DMA wait: wide matmuls early to
            # span the wait, narrow ones at the end for fine granularity
            wa = pers.tile([P, 512], BF16)
            nc.vector.memset(wa, 0.0)
            for i in range(WARM2):
                wacc = psA.tile([P, 512], F32, tag="m1")
                w = 512 if i < WARM2 // 2 else 64
                nc.tensor.matmul(wacc[:, 0:w], wa[:, 0:P], wa[:, 0:w],
                                 start=True, stop=True)

            xl_t = pers.tile([P, 3, CCH, ROWS2], FP8)
            xl_r = [t.rearrange("(c p) t -> p c t", p=P)
                    for t in (xl8, xl8c, xl8d)]
            wh_t = pers.tile([P, CCH, HID], FP8)
            dwh_t = pers.tile([P, CCH, HID], FP8)
            wh_r = wh8.rearrange("(c p) n -> p c n", p=P)
            dwh_r = dwh8.rearrange("(c p) n -> p c n", p=P)
            nc.sync.dma_start(wh_t[:, :, 0:512], wh_r[:, :, 0:512])
            nc.sync.dma_start(xl_t[:, 0], xl_r[0])
            nc.sync.dma_start(dwh_t[:, :, 0:512], dwh_r[:, :, 0:512])
            nc.sync.dma_start(xl_t[:, 1], xl_r[1])
            nc.sync.dma_start(xl_t[:, 2], xl_r[2])
            bh_t = pers.tile([P, 2, HCH], F32)
            nc.sync.dma_start(bh_t, bh[:, :, :])
            for g in range(1, 6):
                sl = slice(g * 512, (g + 1) * 512)
                nc.sync.dma_start(wh_t[:, :, sl], wh_r[:, :, sl])
                nc.sync.dma_start(dwh_t[:, :, sl], dwh_r[:, :, sl])
            wp_t = pers.tile([P, HCH, C], FP8)
            dwp_t = pers.tile([P, HCH, C], FP8)
            wp_r = wp8.rearrange("(h p) n -> p h n", p=P)
            dwp_r = dwp8.rearrange("(h p) n -> p h n", p=P)
            for g in range(6):
                sl = slice(4 * g, 4 * (g + 1))
                nc.sync.dma_start(wp_t[:, sl], wp_r[:, sl])
                nc.sync.dma_start(dwp_t[:, sl], dwp_r[:, sl])

            hid8 = pers.tile([P, HCH, ROWS2], FP8)
            hid8c = pers.tile([P, HCH, ROWS2], FP8)
            out_sb = pers.tile([P, NO, C], BF16)

            for hc in range(HCH):
                hsl = slice(hc * P, (hc + 1) * P)
                acc = psA.tile([P, ROWS2], F32, tag="m1")
                first = True
                for wt, xi in ((wh_t, 0), (dwh_t, 1), (wh_t, 2)):
                    for k in range(3):
                        nc.tensor.matmul(
                            acc, wt[:, 2 * k:2 * k + 2, hsl],
                            xl_t[:, xi, 2 * k:2 * k + 2, :],
                            start=first, stop=(wt is wh_t and xi == 2
                                               and k == 2), perf_mode=DR)
                        first = False
                nc.scalar.activation(hid8[:, hc, :], acc, AF.Relu,
                                     bias=bh_t[:, 0, hc:hc + 1], scale=0.125)
                # correction operand hid8/8 on the otherwise-idle DVE
                # (exact exponent shift of the already-quantized hid8)
                nc.vector.tensor_scalar_mul(hid8c[:, hc, :], hid8[:, hc, :],
                                            0.125)

            # MLP2: chunk-pair-outer so compute streams behind the wp DMAs;
            # all four token blocks accumulate in persistent psum tiles.
            oq_r = oq.rearrange("(o p) c -> p o c", p=P)
            HPR = HCH // 2  # 12 DR chunk-pairs
            ops_a = psB.tile([P, NO, 512], F32, tag="m2a")
            ops_b = psB.tile([P, NO, 256], F32, tag="m2b")
            for j in range(HPR - 1):
                for tb in range(NO):
                    tsl = slice(tb * P, (tb + 1) * P)
                    for ht, wt in ((hid8, wp_t), (hid8c, dwp_t)):
                        st = ht is hid8 and j == 0
                        nc.tensor.matmul(
                            ops_a[:, tb], ht[:, 2 * j:2 * j + 2, tsl],
                            wt[:, 2 * j:2 * j + 2, 0:512],
                            start=st, stop=False, perf_mode=DR,
                            skip_group_check=True)
                        # ops_b regions are half-bank: tb pairs (0,1)/(2,3)
                        # share a psum bank, so only the bank-first tb may
                        # carry start=True (start pending-zeroes the bank)
                        nc.tensor.matmul(
                            ops_b[:, tb], ht[:, 2 * j:2 * j + 2, tsl],
                            wt[:, 2 * j:2 * j + 2, 512:C],
                            start=st and tb % 2 == 0, stop=False,
                            perf_mode=DR, skip_group_check=True)
            j = HPR - 1
            for tb in range(NO):
                tsl = slice(tb * P, (tb + 1) * P)
                for ht, wt in ((hid8, wp_t), (hid8c, dwp_t)):
                    sp = ht is hid8c
                    nc.tensor.matmul(
                        ops_a[:, tb], ht[:, 2 * j:2 * j + 2, tsl],
                        wt[:, 2 * j:2 * j + 2, 0:512],
                        start=False, stop=sp, perf_mode=DR,
                        skip_group_check=True)
                    nc.tensor.matmul(
                        ops_b[:, tb], ht[:, 2 * j:2 * j + 2, tsl],
                        wt[:, 2 * j:2 * j + 2, 512:C],
                        start=False, stop=sp, perf_mode=DR,
                        skip_group_check=True)
                nc.vector.tensor_copy(out_sb[:, tb, 0:512], ops_a[:, tb])
                nc.vector.tensor_copy(out_sb[:, tb, 512:C], ops_b[:, tb])
                nc.sync.dma_start(oq_r[:, tb], out_sb[:, tb])
    nc.compile()
    return nc


def _ln(x, g, b):
    mu = x.mean(-1, keepdims=True)
    var = x.var(-1, keepdims=True)
    return (x - mu) / np.sqrt(var + EPS) * g + b


def _fp8(a):
    return np.ascontiguousarray(a.astype(ml_dtypes.float8_e4m3))


_ident_block = np.vstack(
    [np.eye(P, dtype=np.float32), np.zeros((C - P, P), np.float32)])


def _bf16(a):
    return np.ascontiguousarray(a.astype(ml_dtypes.bfloat16))


def kernel(x, ln1_g, ln1_b, wq, wk, wv, ln2_g, ln2_b, w_hidden, b_hidden,
           w_proj, b_proj):
    x = np.asarray(x, np.float32)
    ln1_g = np.asarray(ln1_g, np.float32)
    ln1_b = np.asarray(ln1_b, np.float32)
    wq = np.asarray(wq, np.float32)
    wk = np.asarray(wk, np.float32)
    wv = np.asarray(wv, np.float32)
    ln2_g = np.asarray(ln2_g, np.float32)
    ln2_b = np.asarray(ln2_b, np.float32)
    w_hidden = np.asarray(w_hidden, np.float32)
    b_hidden = np.asarray(b_hidden, np.float32)
    w_proj = np.asarray(w_proj, np.float32)
    b_proj = np.asarray(b_proj, np.float32)

    trace = bool(int(os.environ.get("KERNEL_TRACE", "0")))
    tkw = dict(trace=True, trace_cores=list(range(8))) if trace else {}

    # ---- host: LN1, transpose to feature-major, quantize ----
    xhat = _ln(x, ln1_g, ln1_b)                        # [B, T, C]
    xhT = [_fp8(xhat[b].T) for b in range(B)]          # [C, T] each

    if "k1" not in _cache:
        _cache["k1"] = build_attn()
    nc1 = _cache["k1"]

    in_maps1 = []
    for core in range(8):
        b, j = divmod(core, NC_PER_B)
        h0 = HG * j
        # col groups: K01, Kh2, Q01, Qh2, I128, V(3 heads)
        wall = _fp8(np.concatenate(
            [wk[h0], wk[h0 + 1], wk[h0 + 2],
             wq[h0], wq[h0 + 1], wq[h0 + 2],
             _ident_block,
             wv[h0], wv[h0 + 1], wv[h0 + 2]], axis=1))
        in_maps1.append({"xhT": xhT[b], "wall": wall})
    r1 = bass_utils.run_bass_kernel_spmd(nc1, in_maps1,
                                         core_ids=list(range(8)), **tkw)

    # ---- host: normalize softmax, assemble heads, residual ----
    attn = np.empty((B, T, C), np.float32)
    for core in range(8):
        b, j = divmod(core, NC_PER_B)
        o = np.asarray(r1.results[core]["oO"]).astype(np.float32)
        o = o[:, :OW].reshape(T, HG, 65)
        attn[b, :, HG * D * j:HG * D * (j + 1)] = \
            (o[:, :, 0:64] / o[:, :, 64:65]).reshape(T, HG * D)
    x_mid = x + attn

    # ---- host: LN2, transpose; launch 2 ----
    h2 = _ln(x_mid, ln2_g, ln2_b).reshape(B * T, C)
    f32 = np.float32
    wh8_q = _fp8(8.0 * w_hidden)
    dwh8_q = _fp8(256.0 * (w_hidden - wh8_q.astype(f32) / 8.0))
    wp8_q = _fp8(8.0 * w_proj)
    dwp8_q = _fp8(64.0 * (w_proj - wp8_q.astype(f32) / 8.0))
    bh_row = b_hidden.reshape(HCH, P).T.astype(f32)
    bh_t = np.ascontiguousarray(
        np.stack([bh_row, bh_row / 8.0], axis=1))

    if "k2" not in _cache:
        _cache["k2"] = build_mlp()
    nc2 = _cache["k2"]

    in_maps2 = []
    for core in range(8):
        rows = slice(core * ROWS2, (core + 1) * ROWS2)
        h2T = np.ascontiguousarray(h2[rows].T)          # [C, ROWS2] f32
        x8 = _fp8(h2T)
        x8c = _fp8(x8.astype(f32) / 32.0)
        x8d = _fp8(h2T - x8.astype(f32))
        in_maps2.append({
            "xl8": x8, "xl8c": x8c, "xl8d": x8d,
            "wh8": wh8_q, "dwh8": dwh8_q,
            "wp8": wp8_q, "dwp8": dwp8_q, "bh": bh_t,
        })
    r2 = bass_utils.run_bass_kernel_spmd(nc2, in_maps2,
                                         core_ids=list(range(8)), **tkw)

    mlp = np.concatenate(
        [np.asarray(r2.results[c]["oq"]).astype(np.float32)
         for c in range(8)], axis=0).reshape(B, T, C)
    out = x_mid + 0.125 * mlp + b_proj[None, None, :]
    if trace:
        _cache["timings"] = [r1.exec_time_ns, r2.exec_time_ns]
        _cache["results"] = [r1, r2]
    return out



# revision 37
# speedup vs baseline: 1.0048x; 1.0048x over previous
"""Trainium2 Bass kernel for a dense pre-LN transformer block.

Shapes (hardcoded): B=2, T=2048, C=768, H=12, D=64, hidden=3072, fp32 I/O.

Strategy (8 NeuronCores, two SPMD launches, host glue between them):
  Launch 1 (attention): core = (batch b in {0,1}) x (head-group of 3 heads).
    Host precomputes LN1(x) (gain/bias applied), transposes it to
    feature-major and quantizes to fp8-e4m3.  Each core: Q/K/V projections
    for its 3 heads as fp8 DoubleRow matmuls (256-row contraction per
    instruction), causal attention in S^T = K @ Q^T layout (keys on
    partitions, so the softmax matrix feeds the A@V matmul as the
    stationary operand).  exp() runs on ScalarE over [128, 3, w] groups
    (all 3 heads of a key-block row in one instruction).  Softmax uses no
    max-subtraction (scores ~ N(0, 0.3)); the denominator comes free from
    a ones-column appended to V.  Output: per-head UNNORMALIZED numerator
    + denominator, bf16; the host divides, assembles heads, adds the
    residual (x_mid = x + attn).
  Launch 2 (MLP): core = 512 contiguous tokens of the flattened [4096, C].
    Host precomputes LN2(x_mid) and fp8 operand splits.  Device: both MLP
    matmuls run as fp8 DoubleRow with power-of-2-scaled error-correction
    terms (W8 = f8(8w) + dW8 = f8(256(w - W8/8)) against x8 / x8/32 / dx8),
    relu emits hid8 and hid8/8 on ScalarE, MLP2 streams chunk-pairs behind
    the weight DMAs.  Host adds x_mid + 0.125*mlp + b_proj.

All heavy math (all matmuls, exp/softmax, relu) runs on device; the host
does input preprocessing (layernorms over inputs / the inter-launch
residual state), sharding, and output assembly.
"""

import os
import sys
import math

for _p in ("/opt/trn_rl_repo", "/root/.axon_site/_ro/trn_rl_repo"):
    if _p not in sys.path and os.path.isdir(_p):
        sys.path.insert(0, _p)

import numpy as np
import ml_dtypes

import concourse.bass as bass
import concourse.mybir as mybir
import concourse.tile as tile
from concourse import bacc
from concourse import bass_utils

BF16 = mybir.dt.bfloat16
F32 = mybir.dt.float32
FP8 = mybir.dt.float8e4
AF = mybir.ActivationFunctionType
DR = mybir.MatmulPerfMode.DoubleRow

B, T, C, H, D = 2, 2048, 768, 12, 64
HID = 4 * C                     # 3072
EPS = 1e-5
SCALE = 1.0 / math.sqrt(C)      # reference scales scores by 1/sqrt(C)
NC_PER_B = 4                    # cores per batch in launch 1
HG = H // NC_PER_B              # heads per core (3)
P = 128
CCH = C // P                    # 6 feature chunks
TBLK = T // P                   # 16 token blocks of 128
ROWS2 = (B * T) // 8            # 512 tokens per core in launch 2
HCH = HID // P                  # 24 hidden chunks
OW = HG * 65                    # 195: per-token attn payload (num|den x 3)
OWP = 256                       # padded to 512B rows for clean DMA
USE_DR_AV = bool(int(os.environ.get("USE_DR_AV", "1")))
WARM1 = 20                      # PE p-state warmup matmuls (launch 1)
WARM2 = 16                      # PE p-state warmup matmuls (launch 2)

_cache = {}


def build_attn():
    """LN'd input (host) -> QKV proj (fp8 DR) -> causal attention."""
    nc = bacc.Bacc("TRN2", target_bir_lowering=False, debug=False,
                   num_devices=8)
    xhT = nc.dram_tensor("xhT", [C, T], FP8, kind="ExternalInput")
    # wall cols: K01 0:128 | Kh2 128:192 | Q01 192:320 | Qh2 320:384 |
    #            I128 384:512 (fp8 identity, rows 0:128) | V 512:704
    wall = nc.dram_tensor("wall", [C, 704], FP8, kind="ExternalInput")
    oO = nc.dram_tensor("oO", [T, OWP], BF16, kind="ExternalOutput")
    dbg = os.environ.get("DEBUG_PT")
    if dbg:
        dpt = nc.dram_tensor("dpt", [P, 4 * HG * 2 * 1024], FP8,
                             kind="ExternalOutput")
        dva = nc.dram_tensor("dva", [P, 8 * HG * 2 * 65], FP8,
                             kind="ExternalOutput")

    with tile.TileContext(nc) as tc:
        with (
            tc.tile_pool(name="pers", bufs=1) as pers,
            tc.tile_pool(name="aux", bufs=2, space="PSUM") as aux,
        ):
            # --- PE warmup: absorb the p-state ramp during the DMA wait ---
            wa = pers.tile([P, 512], BF16)
            nc.vector.memset(wa, 0.0)
            for i in range(WARM1):
                wacc = aux.tile([P, 512], F32, tag="aux")
                w = 512 if i < 6 else 64
                nc.tensor.matmul(wacc[:, 0:w], wa[:, 0:P], wa[:, 0:w],
                                 start=True, stop=True)

            # --- persistent SBUF (QK weights first, then first token
            # quarter, so the first score group starts early) ---
            wall_t = pers.tile([P, CCH, 704], FP8)
            wall_r = wall.rearrange("(c p) f -> p c f", p=P)
            xh_t = pers.tile([P, CCH, T], FP8)
            xh_r = xhT.rearrange("(c p) t -> p c t", p=P)
            nc.sync.dma_start(wall_t[:, :, 0:512], wall_r[:, :, 0:512])
            nc.sync.dma_start(xh_t[:, :, 0:512], xh_r[:, :, 0:512])
            nc.sync.dma_start(wall_t[:, :, 512:704], wall_r[:, :, 512:704])
            for qq in range(1, 4):
                nc.sync.dma_start(xh_t[:, :, qq * 512:(qq + 1) * 512],
                                  xh_r[:, :, qq * 512:(qq + 1) * 512])

            # -240 strict-upper-triangle const: the causal mask is a
            # single psum-accumulated ident^T @ negU, emitted BEFORE the
            # diagonal score matmul so the PE can run it early (it has no
            # data deps) and ScalarE never waits on it.
            # exp((s - 240) * SCALE) < 3e-4 relative leak, only visible in
            # the first few tokens' denominators; -240 is fp8e4-exact.
            ident = wall_t[:, 0, 384:512]
            negU = pers.tile([P, P], FP8)
            nc.gpsimd.memset(negU, 0.0)
            nc.gpsimd.affine_select(
                out=negU, in_=negU, compare_op=mybir.AluOpType.is_ge,
                fill=-240.0, base=0, pattern=[[1, P]], channel_multiplier=-1)

            # V in fp8 with a ones column (denominator), paired key blocks
            # so AV runs as DoubleRow over 256-key contractions
            vaug = pers.tile([P, TBLK // 2, HG, 2, 65], FP8)
            nc.vector.memset(vaug[:, :, :, :, 64:65], 1.0)

            # QKT[p, s, 0, t] = Q features, QKT[p, s, 1, t] = K features;
            # head h lives at partitions 64*(h%2).. with slot s = h//2, so
            # each head's Q and K share a physical partition range (the
            # scores matmul requires equal base partitions).
            QKT = pers.tile([P, 2, 2, T], BF16)
            # probs (exp output) in fp8, paired-key-block layout; separate
            # arrays per query half so tt1 groups never overwrite blocks
            # the tt0 AVs still read
            pt0 = pers.tile([P, 4, HG, 2, 1024], FP8)
            pt1 = pers.tile([P, 8, HG, 2, 1024], FP8)
            o_store = pers.tile([P, TBLK, OWP], BF16)

            # wall col groups: K01@0:128, Kh2@128:192, Q01@192:320,
            # Qh2@320:384, V@384:576
            QK_GROUPS = [  # (col0, width, slot, qk)
                (192, P, 0, 0), (0, P, 0, 1),
                (320, 64, 1, 0), (128, 64, 1, 1),
            ]

            def qk_proj(tch, order=(0, 1, 2, 3)):
                for gi in order:
                    col0, gw, sl, qk = QK_GROUPS[gi]
                    acc = aux.tile([P, 512], F32, tag="aux")
                    for k in range(3):
                        nc.tensor.matmul(
                            acc[0:gw],
                            wall_t[:, 2 * k:2 * k + 2, col0:col0 + gw],
                            xh_t[:, 2 * k:2 * k + 2,
                                 tch * 512:(tch + 1) * 512],
                            start=(k == 0), stop=(k == 2), perf_mode=DR)
                    nc.vector.tensor_copy(
                        QKT[0:gw, sl, qk, tch * 512:(tch + 1) * 512],
                        acc[0:gw])

            def v_proj(ob):
                acc = aux.tile([P, 512], F32, tag="aux")
                for k in range(3):
                    nc.tensor.matmul(
                        acc[:, 0:192],
                        xh_t[:, 2 * k:2 * k + 2, ob * P:(ob + 1) * P],
                        wall_t[:, 2 * k:2 * k + 2, 512:704],
                        start=(k == 0), stop=(k == 2), perf_mode=DR)
                nc.vector.tensor_copy(
                    vaug[:, ob // 2, :, ob % 2, 0:64],
                    acc[:, 0:192].rearrange("p (h d) -> p h d", h=HG))

            qk_proj(0, order=(3, 2, 1, 0))
            qk_proj(1, order=(0, 1, 2, 3))

            # deferred work to interleave into the score loops (PE has
            # slack while ScalarE exp is the bottleneck); kept small per
            # item so a pop never delays the next score matmuls by much
            deferred = [
                lambda: qk_proj(2, order=(1, 0)),
                lambda: qk_proj(2, order=(3, 2)),
                lambda: qk_proj(3, order=(1, 0)),
                lambda: qk_proj(3, order=(3, 2)),
            ] + [lambda ob=ob: v_proj(ob) for ob in range(8, 16)]

            # Two independent single-buffered score pools (heads 0-1 / head
            # 2) so PE fills one while ScalarE exps the other.
            with (
                tc.tile_pool(name="scA", bufs=1, space="PSUM") as scpA,
                tc.tile_pool(name="scB", bufs=1, space="PSUM") as scpB,
            ):
                o_r = oO.rearrange("(o p) f -> p o f", p=P)

                def scores(sc, hs, tt, kb, off, w, diag):
                    for i, h in enumerate(hs):
                        sl, hsel = divmod(h, 2)
                        pb = 64 * hsel
                        s = 0
                        if diag:
                            # diagonal 128 columns: their own psum group;
                            # masks first (dep-free, hoistable), scores close
                            mdst = sc[:, i, off:off + P] \
                                if len(hs) > 1 else sc[:, off:off + P]
                            q0 = tt * 1024 + off
                            nc.tensor.matmul(
                                mdst, ident, negU, start=True,
                                stop=False, skip_group_check=True)
                            nc.tensor.matmul(
                                mdst,
                                QKT[pb:pb + 64, sl, 1, kb * P:(kb + 1) * P],
                                QKT[pb:pb + 64, sl, 0, q0:q0 + P],
                                start=False, stop=True,
                                skip_group_check=True)
                            s = P
                        while s < w:
                            ww = min(512, w - s)
                            q0 = tt * 1024 + off + s
                            dst = sc[:, i, off + s:off + s + ww] \
                                if len(hs) > 1 else sc[:, off + s:off + s + ww]
                            nc.tensor.matmul(
                                dst,
                                QKT[pb:pb + 64, sl, 1, kb * P:(kb + 1) * P],
                                QKT[pb:pb + 64, sl, 0, q0:q0 + ww],
                                start=True, stop=True)
                            s += ww

                def av_store(gq, oacc):
                    nc.vector.tensor_copy(
                        o_store[:, gq, 0:OW], oacc[:, 0:OW])
                    if gq == 14:
                        nc.sync.dma_start(o_r[:, 12:15, :],
                                          o_store[:, 12:15, :])
                    elif gq == 15:
                        nc.sync.dma_start(o_r[:, 15:16, :],
                                          o_store[:, 15:16, :])
                    elif gq % 4 == 3:
                        nc.sync.dma_start(
                            o_r[:, gq - 3:gq + 1, :],
                            o_store[:, gq - 3:gq + 1, :])

                def av_mms(gq, oacc, k2s, last):
                    # paired key blocks run as fp8 DoubleRow (256-key
                    # contraction per matmul); stragglers as single fp8.
                    gl = gq % 8
                    pta = pt0 if gq < 8 else pt1
                    csl = slice(gl * P, (gl + 1) * P)
                    k2s = list(k2s)
                    items, i = [], 0
                    while i < len(k2s):
                        k2 = k2s[i]
                        if USE_DR_AV and k2 % 2 == 0 and i + 1 < len(k2s) \
                                and k2s[i + 1] == k2 + 1:
                            items.append((True, k2 // 2)); i += 2
                        else:
                            items.append((False, k2)); i += 1
                    first_grp = k2s[0] == 0
                    for j, (pair, idx) in enumerate(items):
                        for h in range(HG):
                            st = first_grp and j == 0 and h == 0
                            sp = last and j == len(items) - 1 and h == HG - 1
                            if pair:
                                nc.tensor.matmul(
                                    oacc[:, h * 65:(h + 1) * 65],
                                    pta[:, idx, h, :, csl],
                                    vaug[:, idx, h, :, :],
                                    start=st, stop=sp, perf_mode=DR,
                                    skip_group_check=True)
                            else:
                                nc.tensor.matmul(
                                    oacc[:, h * 65:(h + 1) * 65],
                                    pta[:, idx // 2, h, idx % 2, csl],
                                    vaug[:, idx // 2, h, idx % 2, :],
                                    start=st, stop=sp,
                                    skip_group_check=True)

                def group(tt, kb, fill=0, split=False, b_first=True):
                    off = max(0, P * kb - 1024 * tt)
                    diag = P * kb >= 1024 * tt
                    # (off, width) segments; splitting the first groups at
                    # q=512 lets the exp stream start as soon as the first
                    # xh DMA quarter lands (segment b's data arrives while
                    # ScalarE works on segment a)
                    segs = [(off, 512 - off), (512, 512)] if split \
                        else [(off, 1024 - off)]
                    pta = pt0 if tt == 0 else pt1
                    dstA = pta[:, kb // 2, 0:2, kb % 2, :]
                    dstB = pta[:, kb // 2, 2, kb % 2, :]
                    scA = scpA.tile([P, 2, 1024], F32, tag="scA")
                    scB = scpB.tile([P, 1024], F32, tag="scB")
                    def emit_a(so, sw, si):
                        scores(scA, (0, 1), tt, kb, so, sw, diag and si == 0)
                        nc.scalar.activation(
                            dstA[:, :, so:so + sw], scA[:, :, so:so + sw],
                            AF.Exp, scale=SCALE)

                    def emit_b(so, sw, si):
                        scores(scB, (2,), tt, kb, so, sw, diag and si == 0)
                        nc.scalar.activation(
                            dstB[:, so:so + sw], scB[:, so:so + sw],
                            AF.Exp, scale=SCALE)

                    for si, (so, sw) in enumerate(segs):
                        if b_first:
                            emit_b(so, sw, si)
                            emit_a(so, sw, si)
                        else:
                            emit_a(so, sw, si)
                            emit_b(so, sw, si)
                    # PE filler (runs while ScalarE exps this group); emitted
                    # after the score matmuls so it can't delay them
                    for _ in range(fill):
                        if deferred:
                            deferred.pop(0)()

                def av_full(gq):
                    oacc = aux.tile([P, 512], F32, tag="aux")
                    av_mms(gq, oacc, range(gq + 1), True)
                    av_store(gq, oacc)

                # tt0: ascending kb; AV(gq) emitted one iteration late so
                # it runs inside ScalarE's exp window of the next group.
                # No deferred pops in the first 3 groups (that work needs
                # DMA quarters 3-4 and would stall the PE FIFO).
                for kb in range(8):
                    group(0, kb, fill=0 if kb < 3 else 1, split=(kb < 2),
                          b_first=True)
                    # early V projections land here, behind the first score
                    # matmuls instead of ahead of them in the PE FIFO
                    if 1 <= kb <= 4:
                        v_proj(2 * (kb - 1))
                        v_proj(2 * (kb - 1) + 1)
                    if kb >= 1:
                        av_full(kb - 1)
                # phase boundary: tt1's first group writes pt1, so it can
                # be emitted before the last tt0 AV still reading pt0
                group(1, 0, fill=1)
                av_full(7)

                # tt1: ascending kb, software-pipelined AV: part1 (all key
                # blocks except the diagonal) is emitted right after this
                # group's score matmuls and runs inside ScalarE's exp
                # window; the 3-matmul diagonal part2 + copy-out are emitted
                # after the NEXT group's scores so they never delay them.
                part_acc = {}
                for kb in range(1, 16):
                    group(1, kb, fill=1 if kb < 8 else 0)
                    if kb >= 9:
                        pa = part_acc.pop(kb - 1)
                        av_mms(kb - 1, pa, [kb - 1], True)
                        av_store(kb - 1, pa)
                    if kb >= 8:
                        oacc = aux.tile([P, 512], F32, tag="aux")
                        part_acc[kb] = oacc
                        av_mms(kb, oacc, range(kb), False)
                pa = part_acc.pop(15)
                av_mms(15, pa, [15], True)
                av_store(15, pa)
                if dbg:
                    nc.sync.dma_start(
                        dpt[:, :], pt0.rearrange("p a h j c -> p (a h j c)"))
                    nc.sync.dma_start(
                        dva[:, :], vaug.rearrange("p a h j c -> p (a h j c)"))
    nc.compile()
    return nc


def build_mlp():
    """Host-LN'd x_mid -> MLP1 -> relu -> MLP2 for 512 tokens per core.

    fp8 DoubleRow with power-of-2-scaled error-correction matmuls:
      MLP1 psum (x8 precision, 8x scaled) =
          x8 @ W8 + (x8/32) @ dW8 + dx8 @ W8
      where W8 = f8(8 w), dW8 = f8(256 (w - W8/8)), x8 = f8(h2),
      dx8 = f8(h2 - x8).  relu applies scale 1/8.  hid emitted twice by
      ScalarE: hid8 = f8(relu) and hid8c = f8(relu/8) (corr operand).
      MLP2 psum = hid8 @ Wp8 + hid8c @ dWp8  (8x scaled; host multiplies
      the output by 1/8, exact).
    """
    nc = bacc.Bacc("TRN2", target_bir_lowering=False, debug=False,
                   num_devices=8)
    xl8 = nc.dram_tensor("xl8", [C, ROWS2], FP8, kind="ExternalInput")
    xl8c = nc.dram_tensor("xl8c", [C, ROWS2], FP8, kind="ExternalInput")
    xl8d = nc.dram_tensor("xl8d", [C, ROWS2], FP8, kind="ExternalInput")
    wh8 = nc.dram_tensor("wh8", [C, HID], FP8, kind="ExternalInput")
    dwh8 = nc.dram_tensor("dwh8", [C, HID], FP8, kind="ExternalInput")
    wp8 = nc.dram_tensor("wp8", [HID, C], FP8, kind="ExternalInput")
    dwp8 = nc.dram_tensor("dwp8", [HID, C], FP8, kind="ExternalInput")
    bh = nc.dram_tensor("bh", [P, 2, HCH], F32, kind="ExternalInput")
    oq = nc.dram_tensor("oq", [ROWS2, C], BF16, kind="ExternalOutput")

    NO = ROWS2 // P  # 4 token sub-blocks
    with tile.TileContext(nc) as tc:
        with (
            tc.tile_pool(name="pers", bufs=1) as pers,
            tc.tile_pool(name="psA", bufs=2, space="PSUM") as psA,
            tc.tile_pool(name="psB", bufs=1, space="PSUM") as psB,
        ):
            # PE warmup during the initial DMA wait: wide matmuls early to
            # span the wait, narrow ones at the end for fine granularity
            wa = pers.tile([P, 512], BF16)
            nc.vector.memset(wa, 0.0)
            for i in range(WARM2):
                wacc = psA.tile([P, 512], F32, tag="m1")
                w = 512 if i < WARM2 // 2 else 64
                nc.tensor.matmul(wacc[:, 0:w], wa[:, 0:P], wa[:, 0:w],
                                 start=True, stop=True)

            xl_t = pers.tile([P, 3, CCH, ROWS2], FP8)
            xl_r = [t.rearrange("(c p) t -> p c t", p=P)
                    for t in (xl8, xl8c, xl8d)]
            wh_t = pers.tile([P, CCH, HID], FP8)
            dwh_t = pers.tile([P, CCH, HID], FP8)
            wh_r = wh8.rearrange("(c p) n -> p c n", p=P)
            dwh_r = dwh8.rearrange("(c p) n -> p c n", p=P)
            nc.sync.dma_start(wh_t[:, :, 0:512], wh_r[:, :, 0:512])
            nc.sync.dma_start(xl_t[:, 0], xl_r[0])
            nc.sync.dma_start(dwh_t[:, :, 0:512], dwh_r[:, :, 0:512])
            nc.sync.dma_start(xl_t[:, 1], xl_r[1])
            nc.sync.dma_start(xl_t[:, 2], xl_r[2])
            bh_t = pers.tile([P, 2, HCH], F32)
            nc.sync.dma_start(bh_t, bh[:, :, :])
            for g in range(1, 6):
                sl = slice(g * 512, (g + 1) * 512)
                nc.sync.dma_start(wh_t[:, :, sl], wh_r[:, :, sl])
                nc.sync.dma_start(dwh_t[:, :, sl], dwh_r[:, :, sl])
            wp_t = pers.tile([P, HCH, C], FP8)
            dwp_t = pers.tile([P, HCH, C], FP8)
            wp_r = wp8.rearrange("(h p) n -> p h n", p=P)
            dwp_r = dwp8.rearrange("(h p) n -> p h n", p=P)
            for g in range(6):
                sl = slice(4 * g, 4 * (g + 1))
                nc.sync.dma_start(wp_t[:, sl], wp_r[:, sl])
                nc.sync.dma_start(dwp_t[:, sl], dwp_r[:, sl])

            hid8 = pers.tile([P, HCH, ROWS2], FP8)
            hid8c = pers.tile([P, HCH, ROWS2], FP8)
            out_sb = pers.tile([P, NO, C], BF16)

            for hc in range(HCH):
                hsl = slice(hc * P, (hc + 1) * P)
                acc = psA.tile([P, ROWS2], F32, tag="m1")
                first = True
                for wt, xi in ((wh_t, 0), (dwh_t, 1), (wh_t, 2)):
                    for k in range(3):
                        nc.tensor.matmul(
                            acc, wt[:, 2 * k:2 * k + 2, hsl],
                            xl_t[:, xi, 2 * k:2 * k + 2, :],
                            start=first, stop=(wt is wh_t and xi == 2
                                               and k == 2), perf_mode=DR)
                        first = False
                nc.scalar.activation(hid8[:, hc, :], acc, AF.Relu,
                                     bias=bh_t[:, 0, hc:hc + 1], scale=0.125)
                # correction operand hid8/8 on the otherwise-idle DVE
                # (exact exponent shift of the already-quantized hid8)
                nc.vector.tensor_scalar_mul(hid8c[:, hc, :], hid8[:, hc, :],
                                            0.125)

            # MLP2: chunk-pair-outer so compute streams behind the wp DMAs;
            # all four token blocks accumulate in persistent psum tiles.
            oq_r = oq.rearrange("(o p) c -> p o c", p=P)
            HPR = HCH // 2  # 12 DR chunk-pairs
            ops_a = psB.tile([P, NO, 512], F32, tag="m2a")
            ops_b = psB.tile([P, NO, 256], F32, tag="m2b")
            for j in range(HPR - 1):
                for tb in range(NO):
                    tsl = slice(tb * P, (tb + 1) * P)
                    for ht, wt in ((hid8, wp_t), (hid8c, dwp_t)):
                        st = ht is hid8 and j == 0
                        nc.tensor.matmul(
                            ops_a[:, tb], ht[:, 2 * j:2 * j + 2, tsl],
                            wt[:, 2 * j:2 * j + 2, 0:512],
                            start=st, stop=False, perf_mode=DR,
                            skip_group_check=True)
                        # ops_b regions are half-bank: tb pairs (0,1)/(2,3)
                        # share a psum bank, so only the bank-first tb may
                        # carry start=True (start pending-zeroes the bank)
                        nc.tensor.matmul(
                            ops_b[:, tb], ht[:, 2 * j:2 * j + 2, tsl],
                            wt[:, 2 * j:2 * j + 2, 512:C],
                            start=st and tb % 2 == 0, stop=False,
                            perf_mode=DR, skip_group_check=True)
            j = HPR - 1
            for tb in range(NO):
                tsl = slice(tb * P, (tb + 1) * P)
                for ht, wt in ((hid8, wp_t), (hid8c, dwp_t)):
                    sp = ht is hid8c
                    nc.tensor.matmul(
                        ops_a[:, tb], ht[:, 2 * j:2 * j + 2, tsl],
                        wt[:, 2 * j:2 * j + 2, 0:512],
                        start=False, stop=sp, perf_mode=DR,
                        skip_group_check=True)
                    nc.tensor.matmul(
                        ops_b[:, tb], ht[:, 2 * j:2 * j + 2, tsl],
                        wt[:, 2 * j:2 * j + 2, 512:C],
                        start=False, stop=sp, perf_mode=DR,
                        skip_group_check=True)
                nc.vector.tensor_copy(out_sb[:, tb, 0:512], ops_a[:, tb])
                nc.vector.tensor_copy(out_sb[:, tb, 512:C], ops_b[:, tb])
                nc.sync.dma_start(oq_r[:, tb], out_sb[:, tb])
    nc.compile()
    return nc


def _ln(x, g, b):
    mu = x.mean(-1, keepdims=True)
    var = x.var(-1, keepdims=True)
    return (x - mu) / np.sqrt(var + EPS) * g + b


def _fp8(a):
    return np.ascontiguousarray(a.astype(ml_dtypes.float8_e4m3))


_ident_block = np.vstack(
    [np.eye(P, dtype=np.float32), np.zeros((C - P, P), np.float32)])


def _bf16(a):
    return np.ascontiguousarray(a.astype(ml_dtypes.bfloat16))


def kernel(x, ln1_g, ln1_b, wq, wk, wv, ln2_g, ln2_b, w_hidden, b_hidden,
           w_proj, b_proj):
    x = np.asarray(x, np.float32)
    ln1_g = np.asarray(ln1_g, np.float32)
    ln1_b = np.asarray(ln1_b, np.float32)
    wq = np.asarray(wq, np.float32)
    wk = np.asarray(wk, np.float32)
    wv = np.asarray(wv, np.float32)
    ln2_g = np.asarray(ln2_g, np.float32)
    ln2_b = np.asarray(ln2_b, np.float32)
    w_hidden = np.asarray(w_hidden, np.float32)
    b_hidden = np.asarray(b_hidden, np.float32)
    w_proj = np.asarray(w_proj, np.float32)
    b_proj = np.asarray(b_proj, np.float32)

    trace = bool(int(os.environ.get("KERNEL_TRACE", "0")))
    tkw = dict(trace=True, trace_cores=list(range(8))) if trace else {}

    # ---- host: LN1, transpose to feature-major, quantize ----
    xhat = _ln(x, ln1_g, ln1_b)                        # [B, T, C]
    xhT = [_fp8(xhat[b].T) for b in range(B)]          # [C, T] each

    if "k1" not in _cache:
        _cache["k1"] = build_attn()
    nc1 = _cache["k1"]

    in_maps1 = []
    for core in range(8):
        b, j = divmod(core, NC_PER_B)
        h0 = HG * j
        # col groups: K01, Kh2, Q01, Qh2, I128, V(3 heads)
        wall = _fp8(np.concatenate(
            [wk[h0], wk[h0 + 1], wk[h0 + 2],
             wq[h0], wq[h0 + 1], wq[h0 + 2],
             _ident_block,
             wv[h0], wv[h0 + 1], wv[h0 + 2]], axis=1))
        in_maps1.append({"xhT": xhT[b], "wall": wall})
    r1 = bass_utils.run_bass_kernel_spmd(nc1, in_maps1,
                                         core_ids=list(range(8)), **tkw)

    # ---- host: normalize softmax, assemble heads, residual ----
    attn = np.empty((B, T, C), np.float32)
    for core in range(8):
        b, j = divmod(core, NC_PER_B)
        o = np.asarray(r1.results[core]["oO"]).astype(np.float32)
        o = o[:, :OW].reshape(T, HG, 65)
        attn[b, :, HG * D * j:HG * D * (j + 1)] = \
            (o[:, :, 0:64] / o[:, :, 64:65]).reshape(T, HG * D)
    x_mid = x + attn

    # ---- host: LN2, transpose; launch 2 ----
    h2 = _ln(x_mid, ln2_g, ln2_b).reshape(B * T, C)
    f32 = np.float32
    wh8_q = _fp8(8.0 * w_hidden)
    dwh8_q = _fp8(256.0 * (w_hidden - wh8_q.astype(f32) / 8.0))
    wp8_q = _fp8(8.0 * w_proj)
    dwp8_q = _fp8(64.0 * (w_proj - wp8_q.astype(f32) / 8.0))
    bh_row = b_hidden.reshape(HCH, P).T.astype(f32)
    bh_t = np.ascontiguousarray(
        np.stack([bh_row, bh_row / 8.0], axis=1))

    if "k2" not in _cache:
        _cache["k2"] = build_mlp()
    nc2 = _cache["k2"]

    in_maps2 = []
    for core in range(8):
        rows = slice(core * ROWS2, (core + 1) * ROWS2)
        h2T = np.ascontiguousarray(h2[rows].T)          # [C, ROWS2] f32
        x8 = _fp8(h2T)
        x8c = _fp8(x8.astype(f32) / 32.0)
        x8d = _fp8(h2T - x8.astype(f32))
        in_maps2.append({
            "xl8": x8, "xl8c": x8c, "xl8d": x8d,
            "wh8": wh8_q, "dwh8": dwh8_q,
            "wp8": wp8_q, "dwp8": dwp8_q, "bh": bh_t,
        })
    r2 = bass_utils.run_bass_kernel_spmd(nc2, in_maps2,
                                         core_ids=list(range(8)), **tkw)

    mlp = np.concatenate(
        [np.asarray(r2.results[c]["oq"]).astype(np.float32)
         for c in range(8)], axis=0).reshape(B, T, C)
    out = x_mid + 0.125 * mlp + b_proj[None, None, :]
    if trace:
        _cache["timings"] = [r1.exec_time_ns, r2.exec_time_ns]
        _cache["results"] = [r1, r2]
    return out



# revision 39
# speedup vs baseline: 1.0074x; 1.0026x over previous
"""Trainium2 Bass kernel for a dense pre-LN transformer block.

Shapes (hardcoded): B=2, T=2048, C=768, H=12, D=64, hidden=3072, fp32 I/O.

Strategy (8 NeuronCores, two SPMD launches, host glue between them):
  Launch 1 (attention): core = (batch b in {0,1}) x (head-group of 3 heads).
    Host precomputes LN1(x) (gain/bias applied), transposes it to
    feature-major and quantizes to fp8-e4m3.  Each core: Q/K/V projections
    for its 3 heads as fp8 DoubleRow matmuls (256-row contraction per
    instruction), causal attention in S^T = K @ Q^T layout (keys on
    partitions, so the softmax matrix feeds the A@V matmul as the
    stationary operand).  exp() runs on ScalarE over [128, 3, w] groups
    (all 3 heads of a key-block row in one instruction).  Softmax uses no
    max-subtraction (scores ~ N(0, 0.3)); the denominator comes free from
    a ones-column appended to V.  Output: per-head UNNORMALIZED numerator
    + denominator, bf16; the host divides, assembles heads, adds the
    residual (x_mid = x + attn).
  Launch 2 (MLP): core = 512 contiguous tokens of the flattened [4096, C].
    Host precomputes LN2(x_mid) and fp8 operand splits.  Device: both MLP
    matmuls run as fp8 DoubleRow with power-of-2-scaled error-correction
    terms (W8 = f8(8w) + dW8 = f8(256(w - W8/8)) against x8 / x8/32 / dx8),
    relu emits hid8 and hid8/8 on ScalarE, MLP2 streams chunk-pairs behind
    the weight DMAs.  Host adds x_mid + 0.125*mlp + b_proj.

All heavy math (all matmuls, exp/softmax, relu) runs on device; the host
does input preprocessing (layernorms over inputs / the inter-launch
residual state), sharding, and output assembly.
"""

import os
import sys
import math

for _p in ("/opt/trn_rl_repo", "/root/.axon_site/_ro/trn_rl_repo"):
    if _p not in sys.path and os.path.isdir(_p):
        sys.path.insert(0, _p)

import numpy as np
import ml_dtypes

import concourse.bass as bass
import concourse.mybir as mybir
import concourse.tile as tile
from concourse import bacc
from concourse import bass_utils

BF16 = mybir.dt.bfloat16
F32 = mybir.dt.float32
FP8 = mybir.dt.float8e4
AF = mybir.ActivationFunctionType
DR = mybir.MatmulPerfMode.DoubleRow

B, T, C, H, D = 2, 2048, 768, 12, 64
HID = 4 * C                     # 3072
EPS = 1e-5
SCALE = 1.0 / math.sqrt(C)      # reference scales scores by 1/sqrt(C)
NC_PER_B = 4                    # cores per batch in launch 1
HG = H // NC_PER_B              # heads per core (3)
P = 128
CCH = C // P                    # 6 feature chunks
TBLK = T // P                   # 16 token blocks of 128
ROWS2 = (B * T) // 8            # 512 tokens per core in launch 2
HCH = HID // P                  # 24 hidden chunks
OW = HG * 65                    # 195: per-token attn payload (num|den x 3)
OWP = 256                       # padded to 512B rows for clean DMA
USE_DR_AV = bool(int(os.environ.get("USE_DR_AV", "1")))
WARM1 = 20                      # PE p-state warmup matmuls (launch 1)
WARM2 = 16                      # PE p-state warmup matmuls (launch 2)

_cache = {}


def build_attn():
    """LN'd input (host) -> QKV proj (fp8 DR) -> causal attention."""
    nc = bacc.Bacc("TRN2", target_bir_lowering=False, debug=False,
                   num_devices=8)
    xhT = nc.dram_tensor("xhT", [C, T], FP8, kind="ExternalInput")
    # wall cols: K01 0:128 | Kh2 128:192 | Q01 192:320 | Qh2 320:384 |
    #            I128 384:512 (fp8 identity, rows 0:128) | V 512:704
    wall = nc.dram_tensor("wall", [C, 704], FP8, kind="ExternalInput")
    oO = nc.dram_tensor("oO", [T, OWP], BF16, kind="ExternalOutput")
    dbg = os.environ.get("DEBUG_PT")
    if dbg:
        dpt = nc.dram_tensor("dpt", [P, 4 * HG * 2 * 1024], FP8,
                             kind="ExternalOutput")
        dva = nc.dram_tensor("dva", [P, 8 * HG * 2 * 65], FP8,
                             kind="ExternalOutput")

    with tile.TileContext(nc) as tc:
        with (
            tc.tile_pool(name="pers", bufs=1) as pers,
            tc.tile_pool(name="aux", bufs=2, space="PSUM") as aux,
        ):
            # --- PE warmup: absorb the p-state ramp during the DMA wait ---
            wa = pers.tile([P, 512], BF16)
            nc.vector.memset(wa, 0.0)
            for i in range(WARM1):
                wacc = aux.tile([P, 512], F32, tag="aux")
                w = 512 if i < 6 else 64
                nc.tensor.matmul(wacc[:, 0:w], wa[:, 0:P], wa[:, 0:w],
                                 start=True, stop=True)

            # --- persistent SBUF (QK weights first, then first token
            # quarter, so the first score group starts early) ---
            wall_t = pers.tile([P, CCH, 704], FP8)
            wall_r = wall.rearrange("(c p) f -> p c f", p=P)
            xh_t = pers.tile([P, CCH, T], FP8)
            xh_r = xhT.rearrange("(c p) t -> p c t", p=P)
            nc.sync.dma_start(wall_t[:, :, 0:512], wall_r[:, :, 0:512])
            nc.sync.dma_start(xh_t[:, :, 0:512], xh_r[:, :, 0:512])
            nc.sync.dma_start(wall_t[:, :, 512:704], wall_r[:, :, 512:704])
            for qq in range(1, 4):
                nc.sync.dma_start(xh_t[:, :, qq * 512:(qq + 1) * 512],
                                  xh_r[:, :, qq * 512:(qq + 1) * 512])

            # -240 strict-upper-triangle const: the causal mask is a
            # single psum-accumulated ident^T @ negU, emitted BEFORE the
            # diagonal score matmul so the PE can run it early (it has no
            # data deps) and ScalarE never waits on it.
            # exp((s - 240) * SCALE) < 3e-4 relative leak, only visible in
            # the first few tokens' denominators; -240 is fp8e4-exact.
            ident = wall_t[:, 0, 384:512]
            negU = pers.tile([P, P], FP8)
            nc.gpsimd.memset(negU, 0.0)
            nc.gpsimd.affine_select(
                out=negU, in_=negU, compare_op=mybir.AluOpType.is_ge,
                fill=-240.0, base=0, pattern=[[1, P]], channel_multiplier=-1)

            # V in fp8 with a ones column (denominator), paired key blocks
            # so AV runs as DoubleRow over 256-key contractions
            vaug = pers.tile([P, TBLK // 2, HG, 2, 65], FP8)
            nc.vector.memset(vaug[:, :, :, :, 64:65], 1.0)

            # QKT[p, s, 0, t] = Q features, QKT[p, s, 1, t] = K features;
            # head h lives at partitions 64*(h%2).. with slot s = h//2, so
            # each head's Q and K share a physical partition range (the
            # scores matmul requires equal base partitions).
            QKT = pers.tile([P, 2, 2, T], BF16)
            # probs (exp output) in fp8, paired-key-block layout; separate
            # arrays per query half so tt1 groups never overwrite blocks
            # the tt0 AVs still read
            pt0 = pers.tile([P, 4, HG, 2, 1024], FP8)
            pt1 = pers.tile([P, 8, HG, 2, 1024], FP8)
            o_store = pers.tile([P, TBLK, OWP], BF16)

            # wall col groups: K01@0:128, Kh2@128:192, Q01@192:320,
            # Qh2@320:384, V@384:576
            QK_GROUPS = [  # (col0, width, slot, qk)
                (192, P, 0, 0), (0, P, 0, 1),
                (320, 64, 1, 0), (128, 64, 1, 1),
            ]

            def qk_proj(tch, order=(0, 1, 2, 3)):
                for gi in order:
                    col0, gw, sl, qk = QK_GROUPS[gi]
                    acc = aux.tile([P, 512], F32, tag="aux")
                    for k in range(3):
                        nc.tensor.matmul(
                            acc[0:gw],
                            wall_t[:, 2 * k:2 * k + 2, col0:col0 + gw],
                            xh_t[:, 2 * k:2 * k + 2,
                                 tch * 512:(tch + 1) * 512],
                            start=(k == 0), stop=(k == 2), perf_mode=DR)
                    nc.vector.tensor_copy(
                        QKT[0:gw, sl, qk, tch * 512:(tch + 1) * 512],
                        acc[0:gw])

            def v_proj(ob):
                acc = aux.tile([P, 512], F32, tag="aux")
                for k in range(3):
                    nc.tensor.matmul(
                        acc[:, 0:192],
                        xh_t[:, 2 * k:2 * k + 2, ob * P:(ob + 1) * P],
                        wall_t[:, 2 * k:2 * k + 2, 512:704],
                        start=(k == 0), stop=(k == 2), perf_mode=DR)
                nc.vector.tensor_copy(
                    vaug[:, ob // 2, :, ob % 2, 0:64],
                    acc[:, 0:192].rearrange("p (h d) -> p h d", h=HG))

            qk_proj(0, order=(3, 2, 1, 0))
            qk_proj(1, order=(0, 1, 2, 3))

            # deferred work to interleave into the score loops (PE has
            # slack while ScalarE exp is the bottleneck); kept small per
            # item so a pop never delays the next score matmuls by much
            deferred = [
                lambda: qk_proj(2, order=(1, 0)),
                lambda: qk_proj(2, order=(3, 2)),
                lambda: qk_proj(3, order=(1, 0)),
                lambda: qk_proj(3, order=(3, 2)),
            ] + [lambda ob=ob: v_proj(ob) for ob in range(8, 16)]

            # Two independent single-buffered score pools (heads 0-1 / head
            # 2) so PE fills one while ScalarE exps the other.
            with (
                tc.tile_pool(name="scA", bufs=1, space="PSUM") as scpA,
                tc.tile_pool(name="scB", bufs=1, space="PSUM") as scpB,
            ):
                o_r = oO.rearrange("(o p) f -> p o f", p=P)

                def scores(sc, hs, tt, kb, off, w, diag):
                    for i, h in enumerate(hs):
                        sl, hsel = divmod(h, 2)
                        pb = 64 * hsel
                        s = 0
                        if diag:
                            # diagonal 128 columns: their own psum group;
                            # masks first (dep-free, hoistable), scores close
                            mdst = sc[:, i, off:off + P] \
                                if len(hs) > 1 else sc[:, off:off + P]
                            q0 = tt * 1024 + off
                            nc.tensor.matmul(
                                mdst, ident, negU, start=True,
                                stop=False, skip_group_check=True)
                            nc.tensor.matmul(
                                mdst,
                                QKT[pb:pb + 64, sl, 1, kb * P:(kb + 1) * P],
                                QKT[pb:pb + 64, sl, 0, q0:q0 + P],
                                start=False, stop=True,
                                skip_group_check=True)
                            s = P
                        while s < w:
                            ww = min(512, w - s)
                            q0 = tt * 1024 + off + s
                            dst = sc[:, i, off + s:off + s + ww] \
                                if len(hs) > 1 else sc[:, off + s:off + s + ww]
                            nc.tensor.matmul(
                                dst,
                                QKT[pb:pb + 64, sl, 1, kb * P:(kb + 1) * P],
                                QKT[pb:pb + 64, sl, 0, q0:q0 + ww],
                                start=True, stop=True)
                            s += ww

                def av_store(gq, oacc):
                    nc.vector.tensor_copy(
                        o_store[:, gq, 0:OW], oacc[:, 0:OW])
                    if gq == 14:
                        nc.sync.dma_start(o_r[:, 12:15, :],
                                          o_store[:, 12:15, :])
                    elif gq == 15:
                        nc.sync.dma_start(o_r[:, 15:16, :],
                                          o_store[:, 15:16, :])
                    elif gq % 4 == 3:
                        nc.sync.dma_start(
                            o_r[:, gq - 3:gq + 1, :],
                            o_store[:, gq - 3:gq + 1, :])

                def av_mms(gq, oacc, k2s, last):
                    # paired key blocks run as fp8 DoubleRow (256-key
                    # contraction per matmul); stragglers as single fp8.
                    gl = gq % 8
                    pta = pt0 if gq < 8 else pt1
                    csl = slice(gl * P, (gl + 1) * P)
                    k2s = list(k2s)
                    items, i = [], 0
                    while i < len(k2s):
                        k2 = k2s[i]
                        if USE_DR_AV and k2 % 2 == 0 and i + 1 < len(k2s) \
                                and k2s[i + 1] == k2 + 1:
                            items.append((True, k2 // 2)); i += 2
                        else:
                            items.append((False, k2)); i += 1
                    first_grp = k2s[0] == 0
                    for j, (pair, idx) in enumerate(items):
                        for h in range(HG):
                            st = first_grp and j == 0 and h == 0
                            sp = last and j == len(items) - 1 and h == HG - 1
                            if pair:
                                nc.tensor.matmul(
                                    oacc[:, h * 65:(h + 1) * 65],
                                    pta[:, idx, h, :, csl],
                                    vaug[:, idx, h, :, :],
                                    start=st, stop=sp, perf_mode=DR,
                                    skip_group_check=True)
                            else:
                                nc.tensor.matmul(
                                    oacc[:, h * 65:(h + 1) * 65],
                                    pta[:, idx // 2, h, idx % 2, csl],
                                    vaug[:, idx // 2, h, idx % 2, :],
                                    start=st, stop=sp,
                                    skip_group_check=True)

                def group(tt, kb, fill=0, split=False, b_first=True):
                    off = max(0, P * kb - 1024 * tt)
                    diag = P * kb >= 1024 * tt
                    # (off, width) segments; splitting the first groups at
                    # q=512 lets the exp stream start as soon as the first
                    # xh DMA quarter lands (segment b's data arrives while
                    # ScalarE works on segment a)
                    segs = [(off, 512 - off), (512, 512)] if split \
                        else [(off, 1024 - off)]
                    pta = pt0 if tt == 0 else pt1
                    dstA = pta[:, kb // 2, 0:2, kb % 2, :]
                    dstB = pta[:, kb // 2, 2, kb % 2, :]
                    scA = scpA.tile([P, 2, 1024], F32, tag="scA")
                    scB = scpB.tile([P, 1024], F32, tag="scB")
                    def emit_a(so, sw, si):
                        scores(scA, (0, 1), tt, kb, so, sw, diag and si == 0)
                        nc.scalar.activation(
                            dstA[:, :, so:so + sw], scA[:, :, so:so + sw],
                            AF.Exp, scale=SCALE)

                    def emit_b(so, sw, si):
                        scores(scB, (2,), tt, kb, so, sw, diag and si == 0)
                        nc.scalar.activation(
                            dstB[:, so:so + sw], scB[:, so:so + sw],
                            AF.Exp, scale=SCALE)

                    for si, (so, sw) in enumerate(segs):
                        if b_first:
                            emit_b(so, sw, si)
                            emit_a(so, sw, si)
                        else:
                            emit_a(so, sw, si)
                            emit_b(so, sw, si)
                    # PE filler (runs while ScalarE exps this group); emitted
                    # after the score matmuls so it can't delay them
                    for _ in range(fill):
                        if deferred:
                            deferred.pop(0)()

                def av_full(gq):
                    oacc = aux.tile([P, 512], F32, tag="aux")
                    av_mms(gq, oacc, range(gq + 1), True)
                    av_store(gq, oacc)

                # tt0: ascending kb; AV(gq) emitted one iteration late so
                # it runs inside ScalarE's exp window of the next group.
                # No deferred pops in the first 3 groups (that work needs
                # DMA quarters 3-4 and would stall the PE FIFO).
                for kb in range(8):
                    group(0, kb, fill=0 if kb < 3 else 1, split=(kb < 2),
                          b_first=True)
                    # early V projections land here, behind the first score
                    # matmuls instead of ahead of them in the PE FIFO
                    if 1 <= kb <= 4:
                        v_proj(2 * (kb - 1))
                        v_proj(2 * (kb - 1) + 1)
                    if kb >= 1:
                        av_full(kb - 1)
                # phase boundary: tt1's first group writes pt1, so it can
                # be emitted before the last tt0 AV still reading pt0
                group(1, 0, fill=1)
                av_full(7)

                # tt1: ascending kb, software-pipelined AV: part1 (all key
                # blocks except the diagonal) is emitted right after this
                # group's score matmuls and runs inside ScalarE's exp
                # window; the 3-matmul diagonal part2 + copy-out are emitted
                # after the NEXT group's scores so they never delay them.
                part_acc = {}
                for kb in range(1, 16):
                    group(1, kb, fill=1 if kb < 8 else 0)
                    if kb >= 9:
                        pa = part_acc.pop(kb - 1)
                        av_mms(kb - 1, pa, [kb - 1], True)
                        av_store(kb - 1, pa)
                    if kb >= 8:
                        oacc = aux.tile([P, 512], F32, tag="aux")
                        part_acc[kb] = oacc
                        av_mms(kb, oacc, range(kb), False)
                pa = part_acc.pop(15)
                av_mms(15, pa, [15], True)
                av_store(15, pa)
                if dbg:
                    nc.sync.dma_start(
                        dpt[:, :], pt0.rearrange("p a h j c -> p (a h j c)"))
                    nc.sync.dma_start(
                        dva[:, :], vaug.rearrange("p a h j c -> p (a h j c)"))
    nc.compile()
    return nc


def build_mlp():
    """Host-LN'd x_mid -> MLP1 -> relu -> MLP2 for 512 tokens per core.

    fp8 DoubleRow with power-of-2-scaled error-correction matmuls:
      MLP1 psum (x8 precision, 8x scaled) =
          x8 @ W8 + (x8/32) @ dW8 + dx8 @ W8
      where W8 = f8(8 w), dW8 = f8(256 (w - W8/8)), x8 = f8(h2),
      dx8 = f8(h2 - x8).  relu applies scale 1/8.  hid emitted twice by
      ScalarE: hid8 = f8(relu) and hid8c = f8(relu/8) (corr operand).
      MLP2 psum = hid8 @ Wp8 + hid8c @ dWp8  (8x scaled; host multiplies
      the output by 1/8, exact).
    """
    nc = bacc.Bacc("TRN2", target_bir_lowering=False, debug=False,
                   num_devices=8)
    xl8 = nc.dram_tensor("xl8", [C, ROWS2], FP8, kind="ExternalInput")
    xl8c = nc.dram_tensor("xl8c", [C, ROWS2], FP8, kind="ExternalInput")
    xl8d = nc.dram_tensor("xl8d", [C, ROWS2], FP8, kind="ExternalInput")
    wh8 = nc.dram_tensor("wh8", [C, HID], FP8, kind="ExternalInput")
    dwh8 = nc.dram_tensor("dwh8", [C, HID], FP8, kind="ExternalInput")
    wp8 = nc.dram_tensor("wp8", [HID, C], FP8, kind="ExternalInput")
    dwp8 = nc.dram_tensor("dwp8", [HID, C], FP8, kind="ExternalInput")
    bh = nc.dram_tensor("bh", [P, 2, HCH], F32, kind="ExternalInput")
    oq = nc.dram_tensor("oq", [ROWS2, C], BF16, kind="ExternalOutput")

    NO = ROWS2 // P  # 4 token sub-blocks
    with tile.TileContext(nc) as tc:
        with (
            tc.tile_pool(name="pers", bufs=1) as pers,
            tc.tile_pool(name="psA", bufs=2, space="PSUM") as psA,
            tc.tile_pool(name="psB", bufs=1, space="PSUM") as psB,
        ):
            # PE warmup during the initial DMA wait: wide matmuls early to
            # span the wait, narrow ones at the end for fine granularity
            wa = pers.tile([P, 512], BF16)
            nc.vector.memset(wa, 0.0)
            for i in range(WARM2):
                wacc = psA.tile([P, 512], F32, tag="m1")
                w = 512 if i < WARM2 // 2 else 64
                nc.tensor.matmul(wacc[:, 0:w], wa[:, 0:P], wa[:, 0:w],
                                 start=True, stop=True)

            xl_t = pers.tile([P, 3, CCH, ROWS2], FP8)
            xl_r = [t.rearrange("(c p) t -> p c t", p=P)
                    for t in (xl8, xl8c, xl8d)]
            wh_t = pers.tile([P, CCH, HID], FP8)
            dwh_t = pers.tile([P, CCH, HID], FP8)
            wh_r = wh8.rearrange("(c p) n -> p c n", p=P)
            dwh_r = dwh8.rearrange("(c p) n -> p c n", p=P)
            nc.sync.dma_start(wh_t[:, :, 0:512], wh_r[:, :, 0:512])
            nc.sync.dma_start(xl_t[:, 0], xl_r[0])
            nc.sync.dma_start(dwh_t[:, :, 0:512], dwh_r[:, :, 0:512])
            nc.sync.dma_start(xl_t[:, 1], xl_r[1])
            nc.sync.dma_start(xl_t[:, 2], xl_r[2])
            bh_t = pers.tile([P, 2, HCH], F32)
            nc.sync.dma_start(bh_t, bh[:, :, :])
            for g in range(1, 6):
                sl = slice(g * 512, (g + 1) * 512)
                nc.sync.dma_start(wh_t[:, :, sl], wh_r[:, :, sl])
                nc.sync.dma_start(dwh_t[:, :, sl], dwh_r[:, :, sl])
            wp_t = pers.tile([P, HCH, C], FP8)
            dwp_t = pers.tile([P, HCH, C], FP8)
            wp_r = wp8.rearrange("(h p) n -> p h n", p=P)
            dwp_r = dwp8.rearrange("(h p) n -> p h n", p=P)
            for g in range(6):
                sl = slice(4 * g, 4 * (g + 1))
                nc.sync.dma_start(wp_t[:, sl], wp_r[:, sl])
                nc.sync.dma_start(dwp_t[:, sl], dwp_r[:, sl])

            hid8 = pers.tile([P, HCH, ROWS2], FP8)
            hid8c = pers.tile([P, HCH, ROWS2], FP8)
            out_sb = pers.tile([P, NO, C], BF16)

            for hc in range(HCH):
                hsl = slice(hc * P, (hc + 1) * P)
                acc = psA.tile([P, ROWS2], F32, tag="m1")
                first = True
                for wt, xi in ((wh_t, 0), (dwh_t, 1), (wh_t, 2)):
                    for k in range(3):
                        nc.tensor.matmul(
                            acc, wt[:, 2 * k:2 * k + 2, hsl],
                            xl_t[:, xi, 2 * k:2 * k + 2, :],
                            start=first, stop=(wt is wh_t and xi == 2
                                               and k == 2), perf_mode=DR)
                        first = False
                nc.scalar.activation(hid8[:, hc, :], acc, AF.Relu,
                                     bias=bh_t[:, 0, hc:hc + 1], scale=0.125)
                # correction operand hid8/8 on the otherwise-idle DVE
                # (exact exponent shift of the already-quantized hid8)
                nc.vector.tensor_scalar_mul(hid8c[:, hc, :], hid8[:, hc, :],
                                            0.125)

            # MLP2: chunk-pair-outer so compute streams behind the wp DMAs;
            # all four token blocks accumulate in persistent psum tiles.
            oq_r = oq.rearrange("(o p) c -> p o c", p=P)
            HPR = HCH // 2  # 12 DR chunk-pairs
            # separate psum tiles per token block so a later tb's matmuls
            # never WAR-wait on an earlier tb's copy-out (tile-granularity)
            ops_a = [psB.tile([P, 512], F32, tag=f"m2a{tb}",
                              name=f"opsa{tb}") for tb in range(NO)]
            ops_b2 = [psB.tile([P, 2, 256], F32, tag=f"m2b{g}",
                               name=f"opsb{g}") for g in range(NO // 2)]
            for j in range(HPR - 1):
                for tb in range(NO):
                    tsl = slice(tb * P, (tb + 1) * P)
                    for ht, wt in ((hid8, wp_t), (hid8c, dwp_t)):
                        st = ht is hid8 and j == 0
                        nc.tensor.matmul(
                            ops_a[tb], ht[:, 2 * j:2 * j + 2, tsl],
                            wt[:, 2 * j:2 * j + 2, 0:512],
                            start=st, stop=False, perf_mode=DR,
                            skip_group_check=True)
                        # ops_b regions are half-bank: tb pairs (0,1)/(2,3)
                        # share a psum bank, so only the bank-first tb may
                        # carry start=True (start pending-zeroes the bank)
                        nc.tensor.matmul(
                            ops_b2[tb // 2][:, tb % 2],
                            ht[:, 2 * j:2 * j + 2, tsl],
                            wt[:, 2 * j:2 * j + 2, 512:C],
                            start=st and tb % 2 == 0, stop=False,
                            perf_mode=DR, skip_group_check=True)
            j = HPR - 1
            for tb in range(NO):
                tsl = slice(tb * P, (tb + 1) * P)
                for ht, wt in ((hid8, wp_t), (hid8c, dwp_t)):
                    sp = ht is hid8c
                    nc.tensor.matmul(
                        ops_a[tb], ht[:, 2 * j:2 * j + 2, tsl],
                        wt[:, 2 * j:2 * j + 2, 0:512],
                        start=False, stop=sp, perf_mode=DR,
                        skip_group_check=True)
                    nc.tensor.matmul(
                        ops_b2[tb // 2][:, tb % 2],
                        ht[:, 2 * j:2 * j + 2, tsl],
                        wt[:, 2 * j:2 * j + 2, 512:C],
                        start=False, stop=sp, perf_mode=DR,
                        skip_group_check=True)
                nc.vector.tensor_copy(out_sb[:, tb, 0:512], ops_a[tb])
                nc.vector.tensor_copy(out_sb[:, tb, 512:C],
                                      ops_b2[tb // 2][:, tb % 2])
                nc.sync.dma_start(oq_r[:, tb], out_sb[:, tb])
    nc.compile()
    return nc


def _ln(x, g, b):
    mu = x.mean(-1, keepdims=True)
    var = x.var(-1, keepdims=True)
    return (x - mu) / np.sqrt(var + EPS) * g + b


def _fp8(a):
    return np.ascontiguousarray(a.astype(ml_dtypes.float8_e4m3))


_ident_block = np.vstack(
    [np.eye(P, dtype=np.float32), np.zeros((C - P, P), np.float32)])


def _bf16(a):
    return np.ascontiguousarray(a.astype(ml_dtypes.bfloat16))


def kernel(x, ln1_g, ln1_b, wq, wk, wv, ln2_g, ln2_b, w_hidden, b_hidden,
           w_proj, b_proj):
    x = np.asarray(x, np.float32)
    ln1_g = np.asarray(ln1_g, np.float32)
    ln1_b = np.asarray(ln1_b, np.float32)
    wq = np.asarray(wq, np.float32)
    wk = np.asarray(wk, np.float32)
    wv = np.asarray(wv, np.float32)
    ln2_g = np.asarray(ln2_g, np.float32)
    ln2_b = np.asarray(ln2_b, np.float32)
    w_hidden = np.asarray(w_hidden, np.float32)
    b_hidden = np.asarray(b_hidden, np.float32)
    w_proj = np.asarray(w_proj, np.float32)
    b_proj = np.asarray(b_proj, np.float32)

    trace = bool(int(os.environ.get("KERNEL_TRACE", "0")))
    tkw = dict(trace=True, trace_cores=list(range(8))) if trace else {}

    # ---- host: LN1, transpose to feature-major, quantize ----
    xhat = _ln(x, ln1_g, ln1_b)                        # [B, T, C]
    xhT = [_fp8(xhat[b].T) for b in range(B)]          # [C, T] each

    if "k1" not in _cache:
        _cache["k1"] = build_attn()
    nc1 = _cache["k1"]

    in_maps1 = []
    for core in range(8):
        b, j = divmod(core, NC_PER_B)
        h0 = HG * j
        # col groups: K01, Kh2, Q01, Qh2, I128, V(3 heads)
        wall = _fp8(np.concatenate(
            [wk[h0], wk[h0 + 1], wk[h0 + 2],
             wq[h0], wq[h0 + 1], wq[h0 + 2],
             _ident_block,
             wv[h0], wv[h0 + 1], wv[h0 + 2]], axis=1))
        in_maps1.append({"xhT": xhT[b], "wall": wall})
    r1 = bass_utils.run_bass_kernel_spmd(nc1, in_maps1,
                                         core_ids=list(range(8)), **tkw)

    # ---- host: normalize softmax, assemble heads, residual ----
    attn = np.empty((B, T, C), np.float32)
    for core in range(8):
        b, j = divmod(core, NC_PER_B)
        o = np.asarray(r1.results[core]["oO"]).astype(np.float32)
        o = o[:, :OW].reshape(T, HG, 65)
        attn[b, :, HG * D * j:HG * D * (j + 1)] = \
            (o[:, :, 0:64] / o[:, :, 64:65]).reshape(T, HG * D)
    x_mid = x + attn

    # ---- host: LN2, transpose; launch 2 ----
    h2 = _ln(x_mid, ln2_g, ln2_b).reshape(B * T, C)
    f32 = np.float32
    wh8_q = _fp8(8.0 * w_hidden)
    dwh8_q = _fp8(256.0 * (w_hidden - wh8_q.astype(f32) / 8.0))
    wp8_q = _fp8(8.0 * w_proj)
    dwp8_q = _fp8(64.0 * (w_proj - wp8_q.astype(f32) / 8.0))
    bh_row = b_hidden.reshape(HCH, P).T.astype(f32)
    bh_t = np.ascontiguousarray(
        np.stack([bh_row, bh_row / 8.0], axis=1))

    if "k2" not in _cache:
        _cache["k2"] = build_mlp()
    nc2 = _cache["k2"]

    in_maps2 = []
    for core in range(8):
        rows = slice(core * ROWS2, (core + 1) * ROWS2)
        h2T = np.ascontiguousarray(h2[rows].T)          # [C, ROWS2] f32
        x8 = _fp8(h2T)
        x8c = _fp8(x8.astype(f32) / 32.0)
        x8d = _fp8(h2T - x8.astype(f32))
        in_maps2.append({
            "xl8": x8, "xl8c": x8c, "xl8d": x8d,
            "wh8": wh8_q, "dwh8": dwh8_q,
            "wp8": wp8_q, "dwp8": dwp8_q, "bh": bh_t,
        })
    r2 = bass_utils.run_bass_kernel_spmd(nc2, in_maps2,
                                         core_ids=list(range(8)), **tkw)

    mlp = np.concatenate(
        [np.asarray(r2.results[c]["oq"]).astype(np.float32)
         for c in range(8)], axis=0).reshape(B, T, C)
    out = x_mid + 0.125 * mlp + b_proj[None, None, :]
    if trace:
        _cache["timings"] = [r1.exec_time_ns, r2.exec_time_ns]
        _cache["results"] = [r1, r2]
    return out



# revision 40
# speedup vs baseline: 1.0498x; 1.0421x over previous
"""Trainium2 Bass kernel for a dense pre-LN transformer block.

Shapes (hardcoded): B=2, T=2048, C=768, H=12, D=64, hidden=3072, fp32 I/O.

Strategy (8 NeuronCores, two SPMD launches, host glue between them):
  Launch 1 (attention): core = (batch b in {0,1}) x (head-group of 3 heads).
    Host precomputes LN1(x) (gain/bias applied), transposes it to
    feature-major and quantizes to fp8-e4m3.  Each core: Q/K/V projections
    for its 3 heads as fp8 DoubleRow matmuls (256-row contraction per
    instruction), causal attention in S^T = K @ Q^T layout (keys on
    partitions, so the softmax matrix feeds the A@V matmul as the
    stationary operand).  exp() runs on ScalarE over [128, 3, w] groups
    (all 3 heads of a key-block row in one instruction).  Softmax uses no
    max-subtraction (scores ~ N(0, 0.3)); the denominator comes free from
    a ones-column appended to V.  Output: per-head UNNORMALIZED numerator
    + denominator, bf16; the host divides, assembles heads, adds the
    residual (x_mid = x + attn).
  Launch 2 (MLP): core = 512 contiguous tokens of the flattened [4096, C].
    Host precomputes LN2(x_mid) and fp8 operand splits.  Device: both MLP
    matmuls run as fp8 DoubleRow with power-of-2-scaled error-correction
    terms (W8 = f8(8w) + dW8 = f8(256(w - W8/8)) against x8 / x8/32 / dx8),
    relu emits hid8 and hid8/8 on ScalarE, MLP2 streams chunk-pairs behind
    the weight DMAs.  Host adds x_mid + 0.125*mlp + b_proj.

All heavy math (all matmuls, exp/softmax, relu) runs on device; the host
does input preprocessing (layernorms over inputs / the inter-launch
residual state), sharding, and output assembly.
"""

import os
import sys
import math

for _p in ("/opt/trn_rl_repo", "/root/.axon_site/_ro/trn_rl_repo"):
    if _p not in sys.path and os.path.isdir(_p):
        sys.path.insert(0, _p)

import numpy as np
import ml_dtypes

import concourse.bass as bass
import concourse.mybir as mybir
import concourse.tile as tile
from concourse import bacc
from concourse import bass_utils

BF16 = mybir.dt.bfloat16
F32 = mybir.dt.float32
FP8 = mybir.dt.float8e4
AF = mybir.ActivationFunctionType
DR = mybir.MatmulPerfMode.DoubleRow

B, T, C, H, D = 2, 2048, 768, 12, 64
HID = 4 * C                     # 3072
EPS = 1e-5
SCALE = 1.0 / math.sqrt(C)      # reference scales scores by 1/sqrt(C)
NC_PER_B = 4                    # cores per batch in launch 1
HG = H // NC_PER_B              # heads per core (3)
P = 128
CCH = C // P                    # 6 feature chunks
TBLK = T // P                   # 16 token blocks of 128
ROWS2 = (B * T) // 8            # 512 tokens per core in launch 2
HCH = HID // P                  # 24 hidden chunks
OW = HG * 65                    # 195: per-token attn payload (num|den x 3)
OWP = 256                       # padded to 512B rows for clean DMA
USE_DR_AV = bool(int(os.environ.get("USE_DR_AV", "1")))
WARM1 = 20                      # PE p-state warmup matmuls (launch 1)
WARM2 = 16                      # PE p-state warmup matmuls (launch 2)

_cache = {}


def build_attn():
    """LN'd input (host) -> QKV proj (fp8 DR) -> causal attention."""
    nc = bacc.Bacc("TRN2", target_bir_lowering=False, debug=False,
                   num_devices=8)
    xhT = nc.dram_tensor("xhT", [C, T], FP8, kind="ExternalInput")
    # wall cols: K01 0:128 | Kh2 128:192 | Q01 192:320 | Qh2 320:384 |
    #            I128 384:512 (fp8 identity, rows 0:128) | V 512:704
    wall = nc.dram_tensor("wall", [C, 704], FP8, kind="ExternalInput")
    oO = nc.dram_tensor("oO", [T, OWP], BF16, kind="ExternalOutput")
    dbg = os.environ.get("DEBUG_PT")
    if dbg:
        dpt = nc.dram_tensor("dpt", [P, 4 * HG * 2 * 1024], FP8,
                             kind="ExternalOutput")
        dva = nc.dram_tensor("dva", [P, 8 * HG * 2 * 65], FP8,
                             kind="ExternalOutput")

    with tile.TileContext(nc) as tc:
        with (
            tc.tile_pool(name="pers", bufs=1) as pers,
            tc.tile_pool(name="aux", bufs=2, space="PSUM") as aux,
        ):
            # --- PE warmup: absorb the p-state ramp during the DMA wait ---
            wa = pers.tile([P, 512], BF16)
            nc.vector.memset(wa, 0.0)
            for i in range(WARM1):
                wacc = aux.tile([P, 512], F32, tag="aux")
                w = 512 if i < 6 else 64
                nc.tensor.matmul(wacc[:, 0:w], wa[:, 0:P], wa[:, 0:w],
                                 start=True, stop=True)

            # --- persistent SBUF (QK weights first, then first token
            # quarter, so the first score group starts early) ---
            wall_t = pers.tile([P, CCH, 704], FP8)
            wall_r = wall.rearrange("(c p) f -> p c f", p=P)
            xh_t = pers.tile([P, CCH, T], FP8)
            xh_r = xhT.rearrange("(c p) t -> p c t", p=P)
            nc.sync.dma_start(wall_t[:, :, 0:512], wall_r[:, :, 0:512])
            nc.sync.dma_start(xh_t[:, :, 0:512], xh_r[:, :, 0:512])
            nc.sync.dma_start(wall_t[:, :, 512:704], wall_r[:, :, 512:704])
            for qq in range(1, 4):
                nc.sync.dma_start(xh_t[:, :, qq * 512:(qq + 1) * 512],
                                  xh_r[:, :, qq * 512:(qq + 1) * 512])

            # -240 strict-upper-triangle const: the causal mask is a
            # single psum-accumulated ident^T @ negU, emitted BEFORE the
            # diagonal score matmul so the PE can run it early (it has no
            # data deps) and ScalarE never waits on it.
            # exp((s - 240) * SCALE) < 3e-4 relative leak, only visible in
            # the first few tokens' denominators; -240 is fp8e4-exact.
            ident = wall_t[:, 0, 384:512]
            negU = pers.tile([P, P], FP8)
            nc.gpsimd.memset(negU, 0.0)
            nc.gpsimd.affine_select(
                out=negU, in_=negU, compare_op=mybir.AluOpType.is_ge,
                fill=-240.0, base=0, pattern=[[1, P]], channel_multiplier=-1)

            # V in fp8 with a ones column (denominator), paired key blocks
            # so AV runs as DoubleRow over 256-key contractions
            vaug = pers.tile([P, TBLK // 2, HG, 2, 65], FP8)
            nc.vector.memset(vaug[:, :, :, :, 64:65], 1.0)

            # QKT[p, s, 0, t] = Q features, QKT[p, s, 1, t] = K features;
            # head h lives at partitions 64*(h%2).. with slot s = h//2, so
            # each head's Q and K share a physical partition range (the
            # scores matmul requires equal base partitions).
            QKT = pers.tile([P, 2, 2, T], BF16)
            # probs (exp output) in fp8, paired-key-block layout; separate
            # arrays per query half so tt1 groups never overwrite blocks
            # the tt0 AVs still read
            pt0 = pers.tile([P, 4, HG, 2, 1024], FP8)
            pt1 = pers.tile([P, 8, HG, 2, 1024], FP8)
            o_store = pers.tile([P, TBLK, OWP], BF16)

            # wall col groups: K01@0:128, Kh2@128:192, Q01@192:320,
            # Qh2@320:384, V@384:576
            QK_GROUPS = [  # (col0, width, slot, qk)
                (192, P, 0, 0), (0, P, 0, 1),
                (320, 64, 1, 0), (128, 64, 1, 1),
            ]

            def qk_proj(tch, order=(0, 1, 2, 3)):
                for gi in order:
                    col0, gw, sl, qk = QK_GROUPS[gi]
                    acc = aux.tile([P, 512], F32, tag="aux")
                    for k in range(3):
                        nc.tensor.matmul(
                            acc[0:gw],
                            wall_t[:, 2 * k:2 * k + 2, col0:col0 + gw],
                            xh_t[:, 2 * k:2 * k + 2,
                                 tch * 512:(tch + 1) * 512],
                            start=(k == 0), stop=(k == 2), perf_mode=DR)
                    nc.vector.tensor_copy(
                        QKT[0:gw, sl, qk, tch * 512:(tch + 1) * 512],
                        acc[0:gw])

            def v_proj(ob):
                acc = aux.tile([P, 512], F32, tag="aux")
                for k in range(3):
                    nc.tensor.matmul(
                        acc[:, 0:192],
                        xh_t[:, 2 * k:2 * k + 2, ob * P:(ob + 1) * P],
                        wall_t[:, 2 * k:2 * k + 2, 512:704],
                        start=(k == 0), stop=(k == 2), perf_mode=DR)
                nc.vector.tensor_copy(
                    vaug[:, ob // 2, :, ob % 2, 0:64],
                    acc[:, 0:192].rearrange("p (h d) -> p h d", h=HG))

            qk_proj(0, order=(1, 0, 3, 2))
            qk_proj(1, order=(0, 1, 2, 3))

            # deferred work to interleave into the score loops (PE has
            # slack while ScalarE exp is the bottleneck); kept small per
            # item so a pop never delays the next score matmuls by much
            deferred = [
                lambda: qk_proj(2, order=(1, 0)),
                lambda: qk_proj(2, order=(3, 2)),
                lambda: qk_proj(3, order=(1, 0)),
                lambda: qk_proj(3, order=(3, 2)),
            ] + [lambda ob=ob: v_proj(ob) for ob in range(8, 16)]

            # Two independent single-buffered score pools (heads 0-1 / head
            # 2) so PE fills one while ScalarE exps the other.
            with (
                tc.tile_pool(name="scA", bufs=1, space="PSUM") as scpA,
                tc.tile_pool(name="scB", bufs=1, space="PSUM") as scpB,
            ):
                tog = [0]
                o_r = oO.rearrange("(o p) f -> p o f", p=P)

                def scores3(sc, tt, kb, hstart, w, diag):
                    for i in range(3):
                        sl, hsel = divmod(i, 2)
                        pb = 64 * hsel
                        s = 0
                        if diag:
                            # diagonal 128 cols: mask add (dep-free, can
                            # run early) + score matmul close the group
                            mdst = sc[:, i, 0:P]
                            q0 = tt * 1024 + hstart
                            nc.tensor.matmul(
                                mdst, ident, negU, start=True,
                                stop=False, skip_group_check=True)
                            nc.tensor.matmul(
                                mdst,
                                QKT[pb:pb + 64, sl, 1, kb * P:(kb + 1) * P],
                                QKT[pb:pb + 64, sl, 0, q0:q0 + P],
                                start=False, stop=True,
                                skip_group_check=True)
                            s = P
                        while s < w:
                            ww = min(512, w - s)
                            q0 = tt * 1024 + hstart + s
                            nc.tensor.matmul(
                                sc[:, i, s:s + ww],
                                QKT[pb:pb + 64, sl, 1, kb * P:(kb + 1) * P],
                                QKT[pb:pb + 64, sl, 0, q0:q0 + ww],
                                start=True, stop=True)
                            s += ww

                def av_store(gq, oacc):
                    nc.vector.tensor_copy(
                        o_store[:, gq, 0:OW], oacc[:, 0:OW])
                    if gq == 14:
                        nc.sync.dma_start(o_r[:, 12:15, :],
                                          o_store[:, 12:15, :])
                    elif gq == 15:
                        nc.sync.dma_start(o_r[:, 15:16, :],
                                          o_store[:, 15:16, :])
                    elif gq % 4 == 3:
                        nc.sync.dma_start(
                            o_r[:, gq - 3:gq + 1, :],
                            o_store[:, gq - 3:gq + 1, :])

                def av_mms(gq, oacc, k2s, last):
                    # paired key blocks run as fp8 DoubleRow (256-key
                    # contraction per matmul); stragglers as single fp8.
                    gl = gq % 8
                    pta = pt0 if gq < 8 else pt1
                    csl = slice(gl * P, (gl + 1) * P)
                    k2s = list(k2s)
                    items, i = [], 0
                    while i < len(k2s):
                        k2 = k2s[i]
                        if USE_DR_AV and k2 % 2 == 0 and i + 1 < len(k2s) \
                                and k2s[i + 1] == k2 + 1:
                            items.append((True, k2 // 2)); i += 2
                        else:
                            items.append((False, k2)); i += 1
                    first_grp = k2s[0] == 0
                    for j, (pair, idx) in enumerate(items):
                        for h in range(HG):
                            st = first_grp and j == 0 and h == 0
                            sp = last and j == len(items) - 1 and h == HG - 1
                            if pair:
                                nc.tensor.matmul(
                                    oacc[:, h * 65:(h + 1) * 65],
                                    pta[:, idx, h, :, csl],
                                    vaug[:, idx, h, :, :],
                                    start=st, stop=sp, perf_mode=DR,
                                    skip_group_check=True)
                            else:
                                nc.tensor.matmul(
                                    oacc[:, h * 65:(h + 1) * 65],
                                    pta[:, idx // 2, h, idx % 2, csl],
                                    vaug[:, idx // 2, h, idx % 2, :],
                                    start=st, stop=sp,
                                    skip_group_check=True)

                def group(tt, kb, fill=0, split=False, b_first=True):
                    off = max(0, P * kb - 1024 * tt)
                    diag = P * kb >= 1024 * tt
                    pta = pt0 if tt == 0 else pt1
                    halves = []
                    if off < 512:
                        halves.append((off, 512 - off, diag))
                        halves.append((512, 512, False))
                    else:
                        halves.append((off, 1024 - off, diag))
                    for hs_, hw, dg in halves:
                        pool = scpA if tog[0] % 2 == 0 else scpB
                        tog[0] += 1
                        sc = pool.tile([P, 3, 512], F32, tag="sc")
                        scores3(sc, tt, kb, hs_, hw, dg)
                        nc.scalar.activation(
                            pta[:, kb // 2, :, kb % 2, hs_:hs_ + hw],
                            sc[:, :, 0:hw], AF.Exp, scale=SCALE)
                    # PE filler (runs while ScalarE exps this group); emitted
                    # after the score matmuls so it can't delay them
                    for _ in range(fill):
                        if deferred:
                            deferred.pop(0)()

                def av_full(gq):
                    oacc = aux.tile([P, 512], F32, tag="aux")
                    av_mms(gq, oacc, range(gq + 1), True)
                    av_store(gq, oacc)

                # tt0: ascending kb; AV(gq) emitted one iteration late so
                # it runs inside ScalarE's exp window of the next group.
                # No deferred pops in the first 3 groups (that work needs
                # DMA quarters 3-4 and would stall the PE FIFO).
                for kb in range(8):
                    group(0, kb, fill=0 if kb < 3 else 1, split=(kb < 2),
                          b_first=True)
                    # early V projections land here, behind the first score
                    # matmuls instead of ahead of them in the PE FIFO
                    if 1 <= kb <= 4:
                        v_proj(2 * (kb - 1))
                        v_proj(2 * (kb - 1) + 1)
                    if kb >= 1:
                        av_full(kb - 1)
                # phase boundary: tt1's first group writes pt1, so it can
                # be emitted before the last tt0 AV still reading pt0
                group(1, 0, fill=1)
                av_full(7)

                # tt1: ascending kb, software-pipelined AV: part1 (all key
                # blocks except the diagonal) is emitted right after this
                # group's score matmuls and runs inside ScalarE's exp
                # window; the 3-matmul diagonal part2 + copy-out are emitted
                # after the NEXT group's scores so they never delay them.
                part_acc = {}
                for kb in range(1, 16):
                    group(1, kb, fill=1 if kb < 8 else 0)
                    if kb >= 9:
                        pa = part_acc.pop(kb - 1)
                        av_mms(kb - 1, pa, [kb - 1], True)
                        av_store(kb - 1, pa)
                    if kb >= 8:
                        oacc = aux.tile([P, 512], F32, tag="aux")
                        part_acc[kb] = oacc
                        av_mms(kb, oacc, range(kb), False)
                pa = part_acc.pop(15)
                av_mms(15, pa, [15], True)
                av_store(15, pa)
                if dbg:
                    nc.sync.dma_start(
                        dpt[:, :], pt0.rearrange("p a h j c -> p (a h j c)"))
                    nc.sync.dma_start(
                        dva[:, :], vaug.rearrange("p a h j c -> p (a h j c)"))
    nc.compile()
    return nc


def build_mlp():
    """Host-LN'd x_mid -> MLP1 -> relu -> MLP2 for 512 tokens per core.

    fp8 DoubleRow with power-of-2-scaled error-correction matmuls:
      MLP1 psum (x8 precision, 8x scaled) =
          x8 @ W8 + (x8/32) @ dW8 + dx8 @ W8
      where W8 = f8(8 w), dW8 = f8(256 (w - W8/8)), x8 = f8(h2),
      dx8 = f8(h2 - x8).  relu applies scale 1/8.  hid emitted twice by
      ScalarE: hid8 = f8(relu) and hid8c = f8(relu/8) (corr operand).
      MLP2 psum = hid8 @ Wp8 + hid8c @ dWp8  (8x scaled; host multiplies
      the output by 1/8, exact).
    """
    nc = bacc.Bacc("TRN2", target_bir_lowering=False, debug=False,
                   num_devices=8)
    xl8 = nc.dram_tensor("xl8", [C, ROWS2], FP8, kind="ExternalInput")
    xl8c = nc.dram_tensor("xl8c", [C, ROWS2], FP8, kind="ExternalInput")
    xl8d = nc.dram_tensor("xl8d", [C, ROWS2], FP8, kind="ExternalInput")
    wh8 = nc.dram_tensor("wh8", [C, HID], FP8, kind="ExternalInput")
    dwh8 = nc.dram_tensor("dwh8", [C, HID], FP8, kind="ExternalInput")
    wp8 = nc.dram_tensor("wp8", [HID, C], FP8, kind="ExternalInput")
    dwp8 = nc.dram_tensor("dwp8", [HID, C], FP8, kind="ExternalInput")
    bh = nc.dram_tensor("bh", [P, 2, HCH], F32, kind="ExternalInput")
    oq = nc.dram_tensor("oq", [ROWS2, C], BF16, kind="ExternalOutput")

    NO = ROWS2 // P  # 4 token sub-blocks
    with tile.TileContext(nc) as tc:
        with (
            tc.tile_pool(name="pers", bufs=1) as pers,
            tc.tile_pool(name="psA", bufs=2, space="PSUM") as psA,
            tc.tile_pool(name="psB", bufs=1, space="PSUM") as psB,
        ):
            # PE warmup during the initial DMA wait: wide matmuls early to
            # span the wait, narrow ones at the end for fine granularity
            wa = pers.tile([P, 512], BF16)
            nc.vector.memset(wa, 0.0)
            for i in range(WARM2):
                wacc = psA.tile([P, 512], F32, tag="m1")
                w = 512 if i < WARM2 // 2 else 64
                nc.tensor.matmul(wacc[:, 0:w], wa[:, 0:P], wa[:, 0:w],
                                 start=True, stop=True)

            xl_t = pers.tile([P, 3, CCH, ROWS2], FP8)
            xl_r = [t.rearrange("(c p) t -> p c t", p=P)
                    for t in (xl8, xl8c, xl8d)]
            wh_t = pers.tile([P, CCH, HID], FP8)
            dwh_t = pers.tile([P, CCH, HID], FP8)
            wh_r = wh8.rearrange("(c p) n -> p c n", p=P)
            dwh_r = dwh8.rearrange("(c p) n -> p c n", p=P)
            nc.sync.dma_start(wh_t[:, :, 0:512], wh_r[:, :, 0:512])
            nc.sync.dma_start(xl_t[:, 0], xl_r[0])
            nc.sync.dma_start(dwh_t[:, :, 0:512], dwh_r[:, :, 0:512])
            nc.sync.dma_start(xl_t[:, 1], xl_r[1])
            nc.sync.dma_start(xl_t[:, 2], xl_r[2])
            bh_t = pers.tile([P, 2, HCH], F32)
            nc.sync.dma_start(bh_t, bh[:, :, :])
            for g in range(1, 6):
                sl = slice(g * 512, (g + 1) * 512)
                nc.sync.dma_start(wh_t[:, :, sl], wh_r[:, :, sl])
                nc.sync.dma_start(dwh_t[:, :, sl], dwh_r[:, :, sl])
            wp_t = pers.tile([P, HCH, C], FP8)
            dwp_t = pers.tile([P, HCH, C], FP8)
            wp_r = wp8.rearrange("(h p) n -> p h n", p=P)
            dwp_r = dwp8.rearrange("(h p) n -> p h n", p=P)
            for g in range(6):
                sl = slice(4 * g, 4 * (g + 1))
                nc.sync.dma_start(wp_t[:, sl], wp_r[:, sl])
                nc.sync.dma_start(dwp_t[:, sl], dwp_r[:, sl])

            hid8 = pers.tile([P, HCH, ROWS2], FP8)
            hid8c = pers.tile([P, HCH, ROWS2], FP8)
            out_sb = pers.tile([P, NO, C], BF16)

            for hc in range(HCH):
                hsl = slice(hc * P, (hc + 1) * P)
                acc = psA.tile([P, ROWS2], F32, tag="m1")
                first = True
                for wt, xi in ((wh_t, 0), (dwh_t, 1), (wh_t, 2)):
                    for k in range(3):
                        nc.tensor.matmul(
                            acc, wt[:, 2 * k:2 * k + 2, hsl],
                            xl_t[:, xi, 2 * k:2 * k + 2, :],
                            start=first, stop=(wt is wh_t and xi == 2
                                               and k == 2), perf_mode=DR)
                        first = False
                nc.scalar.activation(hid8[:, hc, :], acc, AF.Relu,
                                     bias=bh_t[:, 0, hc:hc + 1], scale=0.125)
                # correction operand hid8/8 on the otherwise-idle DVE
                # (exact exponent shift of the already-quantized hid8)
                nc.vector.tensor_scalar_mul(hid8c[:, hc, :], hid8[:, hc, :],
                                            0.125)

            # MLP2: chunk-pair-outer so compute streams behind the wp DMAs;
            # all four token blocks accumulate in persistent psum tiles.
            oq_r = oq.rearrange("(o p) c -> p o c", p=P)
            HPR = HCH // 2  # 12 DR chunk-pairs
            # separate psum tiles per token block so a later tb's matmuls
            # never WAR-wait on an earlier tb's copy-out (tile-granularity)
            ops_a = [psB.tile([P, 512], F32, tag=f"m2a{tb}",
                              name=f"opsa{tb}") for tb in range(NO)]
            ops_b2 = [psB.tile([P, 2, 256], F32, tag=f"m2b{g}",
                               name=f"opsb{g}") for g in range(NO // 2)]
            for j in range(HPR - 1):
                for tb in range(NO):
                    tsl = slice(tb * P, (tb + 1) * P)
                    for ht, wt in ((hid8, wp_t), (hid8c, dwp_t)):
                        st = ht is hid8 and j == 0
                        nc.tensor.matmul(
                            ops_a[tb], ht[:, 2 * j:2 * j + 2, tsl],
                            wt[:, 2 * j:2 * j + 2, 0:512],
                            start=st, stop=False, perf_mode=DR,
                            skip_group_check=True)
                        # ops_b regions are half-bank: tb pairs (0,1)/(2,3)
                        # share a psum bank, so only the bank-first tb may
                        # carry start=True (start pending-zeroes the bank)
                        nc.tensor.matmul(
                            ops_b2[tb // 2][:, tb % 2],
                            ht[:, 2 * j:2 * j + 2, tsl],
                            wt[:, 2 * j:2 * j + 2, 512:C],
                            start=st and tb % 2 == 0, stop=False,
                            perf_mode=DR, skip_group_check=True)
            j = HPR - 1
            for tb in range(NO):
                tsl = slice(tb * P, (tb + 1) * P)
                for ht, wt in ((hid8, wp_t), (hid8c, dwp_t)):
                    sp = ht is hid8c
                    nc.tensor.matmul(
                        ops_a[tb], ht[:, 2 * j:2 * j + 2, tsl],
                        wt[:, 2 * j:2 * j + 2, 0:512],
                        start=False, stop=sp, perf_mode=DR,
                        skip_group_check=True)
                    nc.tensor.matmul(
                        ops_b2[tb // 2][:, tb % 2],
                        ht[:, 2 * j:2 * j + 2, tsl],
                        wt[:, 2 * j:2 * j + 2, 512:C],
                        start=False, stop=sp, perf_mode=DR,
                        skip_group_check=True)
                nc.vector.tensor_copy(out_sb[:, tb, 0:512], ops_a[tb])
                nc.vector.tensor_copy(out_sb[:, tb, 512:C],
                                      ops_b2[tb // 2][:, tb % 2])
                nc.sync.dma_start(oq_r[:, tb], out_sb[:, tb])
    nc.compile()
    return nc


def _ln(x, g, b):
    mu = x.mean(-1, keepdims=True)
    var = x.var(-1, keepdims=True)
    return (x - mu) / np.sqrt(var + EPS) * g + b


def _fp8(a):
    return np.ascontiguousarray(a.astype(ml_dtypes.float8_e4m3))


_ident_block = np.vstack(
    [np.eye(P, dtype=np.float32), np.zeros((C - P, P), np.float32)])


def _bf16(a):
    return np.ascontiguousarray(a.astype(ml_dtypes.bfloat16))


def kernel(x, ln1_g, ln1_b, wq, wk, wv, ln2_g, ln2_b, w_hidden, b_hidden,
           w_proj, b_proj):
    x = np.asarray(x, np.float32)
    ln1_g = np.asarray(ln1_g, np.float32)
    ln1_b = np.asarray(ln1_b, np.float32)
    wq = np.asarray(wq, np.float32)
    wk = np.asarray(wk, np.float32)
    wv = np.asarray(wv, np.float32)
    ln2_g = np.asarray(ln2_g, np.float32)
    ln2_b = np.asarray(ln2_b, np.float32)
    w_hidden = np.asarray(w_hidden, np.float32)
    b_hidden = np.asarray(b_hidden, np.float32)
    w_proj = np.asarray(w_proj, np.float32)
    b_proj = np.asarray(b_proj, np.float32)

    trace = bool(int(os.environ.get("KERNEL_TRACE", "0")))
    tkw = dict(trace=True, trace_cores=list(range(8))) if trace else {}

    # ---- host: LN1, transpose to feature-major, quantize ----
    xhat = _ln(x, ln1_g, ln1_b)                        # [B, T, C]
    xhT = [_fp8(xhat[b].T) for b in range(B)]          # [C, T] each

    if "k1" not in _cache:
        _cache["k1"] = build_attn()
    nc1 = _cache["k1"]

    in_maps1 = []
    for core in range(8):
        b, j = divmod(core, NC_PER_B)
        h0 = HG * j
        # col groups: K01, Kh2, Q01, Qh2, I128, V(3 heads)
        wall = _fp8(np.concatenate(
            [wk[h0], wk[h0 + 1], wk[h0 + 2],
             wq[h0], wq[h0 + 1], wq[h0 + 2],
             _ident_block,
             wv[h0], wv[h0 + 1], wv[h0 + 2]], axis=1))
        in_maps1.append({"xhT": xhT[b], "wall": wall})
    r1 = bass_utils.run_bass_kernel_spmd(nc1, in_maps1,
                                         core_ids=list(range(8)), **tkw)

    # ---- host: normalize softmax, assemble heads, residual ----
    attn = np.empty((B, T, C), np.float32)
    for core in range(8):
        b, j = divmod(core, NC_PER_B)
        o = np.asarray(r1.results[core]["oO"]).astype(np.float32)
        o = o[:, :OW].reshape(T, HG, 65)
        attn[b, :, HG * D * j:HG * D * (j + 1)] = \
            (o[:, :, 0:64] / o[:, :, 64:65]).reshape(T, HG * D)
    x_mid = x + attn

    # ---- host: LN2, transpose; launch 2 ----
    h2 = _ln(x_mid, ln2_g, ln2_b).reshape(B * T, C)
    f32 = np.float32
    wh8_q = _fp8(8.0 * w_hidden)
    dwh8_q = _fp8(256.0 * (w_hidden - wh8_q.astype(f32) / 8.0))
    wp8_q = _fp8(8.0 * w_proj)
    dwp8_q = _fp8(64.0 * (w_proj - wp8_q.astype(f32) / 8.0))
    bh_row = b_hidden.reshape(HCH, P).T.astype(f32)
    bh_t = np.ascontiguousarray(
        np.stack([bh_row, bh_row / 8.0], axis=1))

    if "k2" not in _cache:
        _cache["k2"] = build_mlp()
    nc2 = _cache["k2"]

    in_maps2 = []
    for core in range(8):
        rows = slice(core * ROWS2, (core + 1) * ROWS2)
        h2T = np.ascontiguousarray(h2[rows].T)          # [C, ROWS2] f32
        x8 = _fp8(h2T)
        x8c = _fp8(x8.astype(f32) / 32.0)
        x8d = _fp8(h2T - x8.astype(f32))
        in_maps2.append({
            "xl8": x8, "xl8c": x8c, "xl8d": x8d,
            "wh8": wh8_q, "dwh8": dwh8_q,
            "wp8": wp8_q, "dwp8": dwp8_q, "bh": bh_t,
        })
    r2 = bass_utils.run_bass_kernel_spmd(nc2, in_maps2,
                                         core_ids=list(range(8)), **tkw)

    mlp = np.concatenate(
        [np.asarray(r2.results[c]["oq"]).astype(np.float32)
         for c in range(8)], axis=0).reshape(B, T, C)
    out = x_mid + 0.125 * mlp + b_proj[None, None, :]
    if trace:
        _cache["timings"] = [r1.exec_time_ns, r2.exec_time_ns]
        _cache["results"] = [r1, r2]
    return out

